# revision 1
# baseline (speedup 1.0000x reference)
"""Trainium2 Bass kernel for nn_CausalityChainModel (loss_fn), 8-core SPMD.

Self-contained: takes FULL inputs, shards internally across 8 NeuronCores,
runs one Bass/Tile program via run_bass_kernel_spmd, returns the scalar loss.

Key math (validated numerically against the reference on CPU):
- loss_indep's [n,N,n] residual tensor collapses analytically:
      G[j,i,k] = S[i,k] - S[j,i]S[j,k]/s2[j]
  (S = centered Gram of X_ind), and the masked weighted triple sum reduces to
  a handful of [64,64] matrix products.
- BatchNorm (train-mode, biased var) stats come from raw Gram matrices of the
  layer inputs: E[h] = W1 colsum(x)/N, E[h^2] = diag(W1 G W1^T)/N, G = x^T x.
  So BN+LeakyReLU is one ACT pass: Lrelu(psum*scale + bias).
- Large matmuls in bf16; the X_ind chain in float32r (full-rate, ~1e-3 rel);
  the Grams feeding X_ind-path BN stats in full fp32.
- Sharding: sample axes of z_logits/X/noise_indep split across cores;
  noise_trans (Zp) replicated; NCT candidates (Zs) sharded over j with a
  min-combine in the final AllGather.
- Collectives (AllGather only, queued in order): AG1 gram-z partials (first,
  absorbs the ~70us ncfw cold-start under local compute), AG2 X_ind-h BN stat
  sums, AG3 S-gram/colsum/mse partials + per-i distance mins.
"""
import os
import sys
import types
import contextlib

for _p in ("/opt/trn_rl_repo", "/root/.axon_site"):
    if _p not in sys.path:
        sys.path.insert(0, _p)

import numpy as np
import ml_dtypes

import concourse.bass as bass
import concourse.tile as tile
from concourse import mybir
from concourse.bass_utils import run_bass_kernel_spmd

SIZE, NS, LAT, NOISE, HID, BTR, NIND = 64, 16384, 128, 64, 256, 2048, 8192
NCORES = 8
SH_NS = NS // NCORES      # 2048
SH_NI = NIND // NCORES    # 1024
SH_J = NS // NCORES       # 2048 Zs rows per core
BN_EPS = 1e-5
LRELU = 0.01

f32 = mybir.dt.float32
f32r = mybir.dt.float32r
bf16 = mybir.dt.bfloat16
i32 = mybir.dt.int32
AF = mybir.ActivationFunctionType
ALU = mybir.AluOpType
AX = mybir.AxisListType
bfnp = ml_dtypes.bfloat16

AG1F = (LAT + 1) + 65        # gram partials: z | noise_ind
AG2F = 8                  # sum(h2) x4 chunks, sum(h2^2) x4 chunks
AG3F = 98                 # 0-63 S, 64 colsum, 65 mse, 66-97 dmin (32 cols)
NADD = 66
NI_CH = 16
BIGF = 3.0e38

# constant-blob column maps: name -> (rows, col_start, width)
CBF_MAP = {
    "ident_bf": (128, 0, 128), "gW1T_bf": (128, 128, 256),
    "gW1nat0": (128, 384, 128), "gW1nat1": (128, 512, 128),
    "gW2T_bf0": (128, 640, 64), "gW2T_bf1": (128, 704, 64),
    "tW1T_bf": (64, 768, 256), "tW1nat0": (128, 1024, 64),
    "tW1nat1": (128, 1088, 64), "tW2T_bf0": (128, 1152, 128),
    "tW2T_bf1": (128, 1280, 128), "ones_row": (1, 1408, 128),
    "ones_col": (128, 1536, 1),
}
CBF_W = 1537
C32_MAP = {
    "ident_32": (128, 0, 128), "eye": (64, 128, 64), "offd": (64, 192, 64),
    "L": (64, 256, 64), "LT": (64, 320, 64),
    "g_gam0": (128, 384, 1), "g_gam1": (128, 385, 1),
    "g_bet0": (128, 386, 1), "g_bet1": (128, 387, 1),
    "t_gam0": (128, 388, 1), "t_gam1": (128, 389, 1),
    "t_bet0": (128, 390, 1), "t_bet1": (128, 391, 1),
    "g_b2": (64, 392, 1), "t_b2": (128, 393, 1),
    "ones64": (64, 394, 1), "ones128": (128, 395, 1),
}
C32_W = 396
CFR_MAP = {
    "identr": (128, 0, 128), "gW2T_320": (128, 128, 64),
    "gW2T_321": (128, 192, 64), "gW1T_32": (128, 256, 256),
    "tW1T_32": (64, 512, 256), "tW2T_320": (128, 768, 128),
    "tW2T_321": (128, 896, 128),
}
CFR_W = 1024

_CACHE = {}


def _install_profshim():
    if "antenv.axon_hooks" in sys.modules:
        return
    try:
        import antenv
        mod = types.ModuleType("antenv.axon_hooks")
        mod._hook = None
        mod.set_axon_ntff_profile_hook = lambda h: setattr(mod, "_hook", h)
        mod.get_axon_ntff_profile_hook = lambda: mod._hook
        sys.modules["antenv.axon_hooks"] = mod
        antenv.axon_hooks = mod
        from trn_agent_boot import trn_boot
        so = "/opt/axon/libaxon_pjrt.so"
        if os.path.exists(so):
            mod.set_axon_ntff_profile_hook(trn_boot._ntff_profile_via_ctypes(so))
        import concourse.bass_utils as bu
        bu.upload_artifacts = lambda tmpdir: str(tmpdir)
    except Exception:
        pass


def _split_multi_waits(nc, max_waits=1):
    """This walrus build rejects >1 sem-wait per instruction: move extras onto
    EventSemaphore nops (cheap, non-pipeline-flushing) placed just before."""
    for bb in nc.main_func.blocks:
        new_insts = []
        for inst in bb.instructions:
            si = inst.sync_info
            if si is not None and len(si.on_wait) > max_waits:
                waits = list(si.on_wait)
                extra, keep = waits[:-max_waits], waits[-max_waits:]
                for i in range(0, len(extra), max_waits):
                    d = mybir.InstEventSemaphore(
                        name=f"{inst.name}-wsplit{i}", ins=[], outs=[])
                    d.engine = inst.engine
                    d.sync_info = mybir.SyncInfo(
                        on_wait=list(extra[i:i + max_waits]), on_update=[])
                    new_insts.append(d)
                inst.sync_info = mybir.SyncInfo(
                    on_wait=list(keep), on_update=list(si.on_update))
            new_insts.append(inst)
        try:
            bb.instructions[:] = new_insts
        except TypeError:
            bb.instructions = new_insts


def _build_program():
    nc = bass.Bass()

    def din(name, shape, dt):
        return nc.dram_tensor(name, shape, dt, kind="ExternalInput")

    zext = din("zext", [NS, LAT + 1], bf16)            # gather source
    znat32 = din("znat32", [SH_NS, LAT + 1], f32)      # shard, z|ones fp32
    zT_sh = din("zT_sh", [LAT, SH_NS], bf16)
    xT_sh = din("xT_sh", [SIZE, SH_NS], bf16)
    ntrT = din("ntrT", [NOISE, BTR], bf16)
    ntr_ext = din("ntr_ext", [BTR, NOISE + 1], bf16)
    nind_nat = din("nind_nat", [SH_NI, NOISE + 1], f32)
    nindT32 = din("nindT32", [NOISE, SH_NI], f32r)
    perm_sh = din("perm_sh", [128, NI_CH], i32)
    cbf_d = din("cbf", [128, CBF_W], bf16)
    c32_d = din("c32", [128, C32_W], f32)
    cfr_d = din("cfr", [128, CFR_W], f32r)

    out_d = nc.dram_tensor("out", [1, 1], f32, kind="ExternalOutput")

    ag1_out = nc.dram_tensor("ag1_out", [NCORES * 128, AG1F], f32,
                             addr_space="Shared")
    ag2_out = nc.dram_tensor("ag2_out", [NCORES * 128, AG2F], f32,
                             addr_space="Shared")
    ag3_out = nc.dram_tensor("ag3_out", [NCORES * 128, AG3F], f32,
                             addr_space="Shared")

    with tile.TileContext(nc) as tc, contextlib.ExitStack() as ctx:
        const = ctx.enter_context(tc.tile_pool(name="const", bufs=1))
        sb = ctx.enter_context(tc.tile_pool(name="sb", bufs=1))
        sb3 = ctx.enter_context(tc.tile_pool(name="sb3", bufs=4))
        ps_acc = ctx.enter_context(tc.tile_pool(name="ps_acc", bufs=2, space="PSUM"))
        ps_sm = ctx.enter_context(tc.tile_pool(name="ps_sm", bufs=2, space="PSUM"))
        ps_d = ctx.enter_context(tc.tile_pool(name="ps_d", bufs=2, space="PSUM"))
        dram = ctx.enter_context(tc.tile_pool(name="dram", bufs=1, space="DRAM"))

        # ---------------- input loads (few big DMAs; gram inputs first)
        t_znat = sb.tile([128, SH_NS // 128, LAT + 1], f32, name="t_znat")
        nc.sync.dma_start(out=t_znat[:],
                          in_=znat32[:].rearrange("(c p) f -> p c f", p=128))
        t_nin = sb.tile([128, SH_NI // 128, NOISE + 1], f32, name="t_nin")
        nc.sync.dma_start(out=t_nin[:],
                          in_=nind_nat[:].rearrange("(c p) f -> p c f", p=128))
        t_ntn = sb.tile([128, BTR // 128, NOISE + 1], bf16, name="t_ntn")
        nc.sync.dma_start(out=t_ntn[:],
                          in_=ntr_ext[:].rearrange("(c p) f -> p c f", p=128))
        cbf = const.tile([128, CBF_W], bf16, name="cbf")
        nc.sync.dma_start(out=cbf[:], in_=cbf_d[:])
        c32 = const.tile([128, C32_W], f32, name="c32")
        nc.sync.dma_start(out=c32[:], in_=c32_d[:])
        cfr = const.tile([128, CFR_W], f32r, name="cfr")
        nc.sync.dma_start(out=cfr[:], in_=cfr_d[:])

        def V(blob, m, name):
            r, c0, w = m[name]
            return blob[:r, c0:c0 + w]

        ident_bf = V(cbf, CBF_MAP, "ident_bf")
        gW1T_bf = V(cbf, CBF_MAP, "gW1T_bf")
        gW1nat = [V(cbf, CBF_MAP, f"gW1nat{b}") for b in range(2)]
        gW2T_bf = [V(cbf, CBF_MAP, f"gW2T_bf{b}") for b in range(2)]
        tW1T_bf = V(cbf, CBF_MAP, "tW1T_bf")
        tW1nat = [V(cbf, CBF_MAP, f"tW1nat{b}") for b in range(2)]
        tW2T_bf = [V(cbf, CBF_MAP, f"tW2T_bf{b}") for b in range(2)]
        ones_row = V(cbf, CBF_MAP, "ones_row")
        ones_col = V(cbf, CBF_MAP, "ones_col")
        ident_32 = V(c32, C32_MAP, "ident_32")
        eye = V(c32, C32_MAP, "eye")
        offd = V(c32, C32_MAP, "offd")
        Lc = V(c32, C32_MAP, "L")
        LTc = V(c32, C32_MAP, "LT")
        g_gam = [V(c32, C32_MAP, f"g_gam{b}") for b in range(2)]
        g_bet = [V(c32, C32_MAP, f"g_bet{b}") for b in range(2)]
        t_gam = [V(c32, C32_MAP, f"t_gam{b}") for b in range(2)]
        t_bet = [V(c32, C32_MAP, f"t_bet{b}") for b in range(2)]
        g_b2 = V(c32, C32_MAP, "g_b2")
        t_b2 = V(c32, C32_MAP, "t_b2")
        ones64 = V(c32, C32_MAP, "ones64")
        ones128 = V(c32, C32_MAP, "ones128")
        identr = V(cfr, CFR_MAP, "identr")
        gW2T_32 = [V(cfr, CFR_MAP, f"gW2T_32{b}") for b in range(2)]
        gW1T_32 = V(cfr, CFR_MAP, "gW1T_32")
        tW1T_32 = V(cfr, CFR_MAP, "tW1T_32")
        tW2T_32 = [V(cfr, CFR_MAP, f"tW2T_32{b}") for b in range(2)]
        eps_col = const.tile([128, 1], f32, tag="eps_col", name="eps_col")
        nc.vector.memset(eps_col[:], BN_EPS)

        t_zT = sb.tile([LAT, SH_NS], bf16, name="t_zT")
        nc.sync.dma_start(out=t_zT[:], in_=zT_sh[:])
        t_xT = sb.tile([SIZE, SH_NS], bf16, name="t_xT")
        nc.sync.dma_start(out=t_xT[:], in_=xT_sh[:])
        t_ntrT = sb.tile([NOISE, BTR], bf16, name="t_ntrT")
        nc.sync.dma_start(out=t_ntrT[:], in_=ntrT[:])
        t_nindT = sb.tile([NOISE, SH_NI], f32r, name="t_nindT")
        nc.sync.dma_start(out=t_nindT[:], in_=nindT32[:])
        t_perm = sb.tile([128, NI_CH], i32, name="t_perm")
        nc.sync.dma_start(out=t_perm[:], in_=perm_sh[:])

        # ---------------- AG1: sharded fp32/bf16 input grams (z, n_ind, n_tr)
        pay1 = sb.tile([128, AG1F], f32, name="pay1")
        gz_ps = ps_acc.tile([LAT, LAT + 1], f32, tag="acc", name="gz_ps")
        for k in range(SH_NS // 128):
            nc.tensor.matmul(out=gz_ps[:], lhsT=t_znat[:, k, :LAT],
                             rhs=t_znat[:, k, :],
                             start=(k == 0), stop=(k == SH_NS // 128 - 1))
        nc.scalar.copy(out=pay1[:, 0:LAT + 1], in_=gz_ps[:])
        gni_ps = ps_acc.tile([NOISE, NOISE + 1], f32, tag="acc", name="gni_ps")
        for k in range(SH_NI // 128):
            nc.tensor.matmul(out=gni_ps[:], lhsT=t_nin[:, k, :NOISE],
                             rhs=t_nin[:, k, :],
                             start=(k == 0), stop=(k == SH_NI // 128 - 1))
        nc.scalar.copy(out=pay1[:NOISE, LAT + 1:LAT + 1 + 65], in_=gni_ps[:])
        ag1_in = dram.tile([128, AG1F], f32, name="ag1_in")
        nc.sync.dma_start(out=ag1_in[:], in_=pay1[:])
        nc.gpsimd.collective_compute(
            "AllGather", ALU.bypass, ins=[ag1_in[:].opt()],
            outs=[ag1_out[:].opt()], replica_groups=[list(range(NCORES))])
        ag1l = sb.tile([128, NCORES, AG1F], f32, name="ag1l")
        nc.sync.dma_start(out=ag1l[:],
                          in_=ag1_out[:].rearrange("(c p) f -> p c f", p=128))

        # ---------------- replicated gram of noise_trans (local, feeds Zp now)
        gtr_ps = ps_acc.tile([NOISE, NOISE + 1], f32, tag="acc", name="gtr_ps")
        for k in range(BTR // 128):
            nc.tensor.matmul(out=gtr_ps[:], lhsT=t_ntn[:, k, :NOISE],
                             rhs=t_ntn[:, k, :],
                             start=(k == 0), stop=(k == BTR // 128 - 1))
        gtr_t = sb.tile([NOISE, NOISE + 1], f32, name="gtr_t")
        nc.scalar.copy(out=gtr_t[:], in_=gtr_ps[:])

        # ---------------- Zs gather + transpose + nsq broadcast rows
        zsT = sb.tile([LAT, SH_J], bf16, name="zsT")
        for g in range(NI_CH):
            gz_t = sb3.tile([128, LAT + 1], bf16, tag="zs_gather", name="zs_gather")
            nc.gpsimd.indirect_dma_start(
                out=gz_t[:], out_offset=None, in_=zext[:],
                in_offset=bass.IndirectOffsetOnAxis(ap=t_perm[:, g:g + 1], axis=0))
            tp = ps_sm.tile([128, 128], bf16, tag="sm", name="zs_tp")
            nc.tensor.transpose(out=tp[:], in_=gz_t[:, :LAT], identity=ident_bf[:])
            nc.scalar.copy(out=zsT[:, g * 128:(g + 1) * 128], in_=tp[:])
        zsq = sb.tile([LAT, SH_J], bf16, tag="sq128", name="zsq")
        nc.scalar.activation(out=zsq[:], in_=zsT[:], func=AF.Square)
        nsq_row = sb.tile([1, SH_J], bf16, name="nsq_row")
        for n in range(SH_J // 512):
            np_ = ps_sm.tile([1, 512], f32, tag="sm", name="nsqp")
            nc.tensor.matmul(out=np_[:], lhsT=ones_col[:],
                             rhs=zsq[:, n * 512:(n + 1) * 512],
                             start=True, stop=True)
            nc.scalar.copy(out=nsq_row[:, n * 512:(n + 1) * 512], in_=np_[:])


        # ---------------- BN stats from a Gram
        def _stat_tail(esq_or_tot2, mu, gam, bet, N, tag):
            var = sb.tile([128, 1], f32, tag=f"var_{tag}", name=f"var_{tag}")
            nc.scalar.activation(out=var[:], in_=esq_or_tot2[:], func=AF.Copy,
                                 scale=1.0 / N)
            musq = sb.tile([128, 1], f32, tag="stat_musq", name="stat_musq")
            nc.vector.tensor_tensor(out=musq[:], in0=mu[:], in1=mu[:], op=ALU.mult)
            nc.vector.tensor_tensor(out=var[:], in0=var[:], in1=musq[:],
                                    op=ALU.subtract)
            std = sb.tile([128, 1], f32, tag="stat_std", name="stat_std")
            nc.scalar.activation(out=std[:], in_=var[:], func=AF.Sqrt,
                                 bias=eps_col[:])
            rstd = sb.tile([128, 1], f32, tag="stat_rstd", name="stat_rstd")
            nc.vector.reciprocal(out=rstd[:], in_=std[:])
            s = sb.tile([128, 1], f32, tag=f"s_{tag}", name=f"s_{tag}")
            nc.vector.tensor_tensor(out=s[:], in0=gam[:], in1=rstd[:], op=ALU.mult)
            bb_ = sb.tile([128, 1], f32, tag=f"b_{tag}", name=f"b_{tag}")
            nc.vector.tensor_tensor(out=bb_[:], in0=mu[:], in1=s[:], op=ALU.mult)
            nc.vector.tensor_tensor(out=bb_[:], in0=bet[:], in1=bb_[:],
                                    op=ALU.subtract)
            return s, bb_

        def stats_from_gram(gram, w1T, w1nat, gam, bet, n_in, N, tag,
                            use_bf=True):
            if use_bf:
                gmm = sb.tile([n_in, n_in + 1], bf16, tag=f"gb_{tag}",
                              name=f"gb_{tag}")
                nc.scalar.copy(out=gmm[:], in_=gram)
            else:
                gmm = gram
            scales, biases = [], []
            for b in range(2):
                mm = ps_acc.tile([128, n_in + 1], f32, tag="acc", name="stat_mm")
                nc.tensor.matmul(out=mm[:], lhsT=w1T[:, b * 128:(b + 1) * 128],
                                 rhs=(gmm[:] if hasattr(gmm, 'tile') or hasattr(gmm, 'pool') else gmm),
                                 start=True, stop=True)
                prod = sb.tile([128, n_in], f32, tag="stat_prod", name="stat_prod")
                nc.vector.tensor_tensor(out=prod[:], in0=mm[:, :n_in],
                                        in1=w1nat[b][:], op=ALU.mult)
                esq = sb.tile([128, 1], f32, tag=f"esq_{tag}{b}",
                              name=f"esq_{tag}{b}")
                nc.vector.reduce_sum(out=esq[:], in_=prod[:], axis=AX.X)
                mu = sb.tile([128, 1], f32, tag=f"mu_{tag}{b}", name=f"mu_{tag}{b}")
                nc.scalar.activation(out=mu[:], in_=mm[:, n_in:n_in + 1],
                                     func=AF.Copy, scale=1.0 / N)
                s, bias = _stat_tail(esq, mu, gam[b], bet[b], N, f"{tag}{b}")
                scales.append(s)
                biases.append(bias)
            return scales, biases

        tr_s, tr_b = stats_from_gram(gtr_t[:], tW1T_bf, tW1nat, t_gam, t_bet,
                                     NOISE, BTR, "tr")
        # ---------------- tr branch: Zp (replicated), -2*(Zp+b2)
        h_tr = [sb.tile([128, BTR], bf16, tag=f"h_tr{b}", name=f"h_tr{b}")
                for b in range(2)]
        for b in range(2):
            for n in range(BTR // 512):
                hp = ps_sm.tile([128, 512], f32, tag="sm", name="hmm")
                nc.tensor.matmul(out=hp[:], lhsT=tW1T_bf[:, b * 128:(b + 1) * 128],
                                 rhs=t_ntrT[:, n * 512:(n + 1) * 512],
                                 start=True, stop=True)
                nc.scalar.activation(out=h_tr[b][:, n * 512:(n + 1) * 512],
                                     in_=hp[:], func=AF.Lrelu,
                                     bias=tr_b[b][:], scale=tr_s[b][:],
                                     alpha=LRELU)
        zpm2 = sb.tile([LAT, BTR], bf16, name="zpm2")
        for n in range(BTR // 512):
            zp = ps_sm.tile([LAT, 512], f32, tag="sm", name="zpmm")
            for b in range(2):
                nc.tensor.matmul(out=zp[:], lhsT=tW2T_bf[b][:],
                                 rhs=h_tr[b][:, n * 512:(n + 1) * 512],
                                 start=(b == 0), stop=(b == 1))
            nc.vector.tensor_scalar(out=zpm2[:, n * 512:(n + 1) * 512], in0=zp[:],
                                    scalar1=t_b2[:], scalar2=-2.0,
                                    op0=ALU.add, op1=ALU.mult)
        zpsq_scr = sb.tile([LAT, BTR], bf16, tag="sq128", name="zpsq_scr")
        zpsq_col = sb.tile([128, 1], f32, name="zpsq_col")
        nc.scalar.activation(out=zpsq_scr[:], in_=zpm2[:], func=AF.Square,
                             accum_out=zpsq_col[:])

        # ---------------- NCT distance loop (overlaps AG1/AG2)
        pay3 = sb.tile([128, AG3F], f32, name="pay3")
        nc.vector.memset(pay3[:], 0.0)
        for ic in range(NI_CH // 2):
            for jh in range(2):
                dps = ps_d.tile([128, 1024], f32, tag="dps", name="dps")
                # batch the two K=1 prefills (one LDWEIGHTS), then the two dots
                for jq in range(2):
                    off = jh * 1024 + jq * 512
                    sl = slice(jq * 512, (jq + 1) * 512)
                    nc.tensor.matmul(out=dps[:, sl], lhsT=ones_row[:],
                                     rhs=nsq_row[:, off:off + 512],
                                     start=True, stop=False)
                for jq in range(2):
                    off = jh * 1024 + jq * 512
                    sl = slice(jq * 512, (jq + 1) * 512)
                    nc.tensor.matmul(out=dps[:, sl],
                                     lhsT=zpm2[:, ic * 128:(ic + 1) * 128],
                                     rhs=zsT[:, off:off + 512],
                                     start=False, stop=True)
                col = NADD + ic * 2 + jh
                nc.vector.tensor_reduce(out=pay3[:, col:col + 1], in_=dps[:],
                                        axis=AX.X, op=ALU.min)

        # ---------------- AG1 combine -> full-batch grams
        gz = sb.tile([128, AG1F], f32, name="gz")
        nc.vector.tensor_tensor(out=gz[:], in0=ag1l[:, 0, :], in1=ag1l[:, 1, :],
                                op=ALU.add)
        for c in range(2, NCORES):
            nc.vector.tensor_tensor(out=gz[:], in0=gz[:], in1=ag1l[:, c, :],
                                    op=ALU.add)
        gni = gz[:NOISE, LAT + 1:LAT + 1 + 65]
        gz_g = gz[:, 0:LAT + 1]


        # stats matmuls for the ind path also in bf16 weights but fp32 gram:
        # mixed dtypes are not allowed -> cast gram to bf16 would lose the
        # fp32 gain; instead run these two stat matmuls in fp32.
        ind_s, ind_b = [], []
        for b in range(2):
            mm = ps_acc.tile([128, NOISE + 1], f32, tag="acc", name="istat_mm")
            # fp32 matmul: lhsT fp32 [64, 128], rhs fp32 [64, 65]
            tW1T_f = sb.tile([NOISE, 128], f32, tag=f"tW1Tf{b}", name=f"tW1Tf{b}")
            nc.vector.tensor_copy(out=tW1T_f[:], in_=tW1T_32[:, b * 128:(b + 1) * 128])
            nc.tensor.matmul(out=mm[:], lhsT=tW1T_f[:], rhs=gni,
                             start=True, stop=True)
            prod = sb.tile([128, NOISE], f32, tag="stat_prod", name="stat_prod")
            nc.vector.tensor_tensor(out=prod[:], in0=mm[:, :NOISE],
                                    in1=tW1nat[b][:], op=ALU.mult)
            esq = sb.tile([128, 1], f32, tag=f"esq_ind{b}", name=f"esq_ind{b}")
            nc.vector.reduce_sum(out=esq[:], in_=prod[:], axis=AX.X)
            mu = sb.tile([128, 1], f32, tag=f"mu_ind{b}", name=f"mu_ind{b}")
            nc.scalar.activation(out=mu[:], in_=mm[:, NOISE:NOISE + 1],
                                 func=AF.Copy, scale=1.0 / NIND)
            s, bias = _stat_tail(esq, mu, t_gam[b], t_bet[b], NIND, f"ind{b}")
            ind_s.append(s)
            ind_b.append(bias)

        # ---------------- ind chain (f32r): h_ind -> Z_ind -> h2 (+ stat sums)
        h_ind = [sb.tile([128, SH_NI], f32r, tag=f"h_ind{b}", name=f"h_ind{b}")
                 for b in range(2)]
        for b in range(2):
            for n in range(SH_NI // 512):
                hp = ps_sm.tile([128, 512], f32, tag="sm", name="himm")
                nc.tensor.matmul(out=hp[:], lhsT=tW1T_32[:, b * 128:(b + 1) * 128],
                                 rhs=t_nindT[:, n * 512:(n + 1) * 512],
                                 start=True, stop=True)
                nc.scalar.activation(out=h_ind[b][:, n * 512:(n + 1) * 512],
                                     in_=hp[:], func=AF.Lrelu,
                                     bias=ind_b[b][:], scale=ind_s[b][:],
                                     alpha=LRELU)
        ziT = sb.tile([LAT, SH_NI], f32r, name="ziT")
        for n in range(SH_NI // 512):
            zp = ps_sm.tile([LAT, 512], f32, tag="sm", name="zimm")
            for b in range(2):
                nc.tensor.matmul(out=zp[:], lhsT=tW2T_32[b][:],
                                 rhs=h_ind[b][:, n * 512:(n + 1) * 512],
                                 start=(b == 0), stop=(b == 1))
            nc.vector.tensor_scalar_add(out=ziT[:, n * 512:(n + 1) * 512],
                                        in0=zp[:], scalar1=t_b2[:])
        pay2 = sb.tile([128, AG2F], f32, name="pay2")
        h2 = [sb.tile([128, SH_NI], f32r, tag=f"h2_{b}", name=f"h2_{b}")
              for b in range(2)]
        sq_scr = sb.tile([128, 512], f32, tag="sqscr32", name="sq_scr")
        for b in range(2):
            for n in range(SH_NI // 512):
                hp = ps_sm.tile([128, 512], f32, tag="sm", name="h2mm")
                nc.tensor.matmul(out=hp[:], lhsT=gW1T_32[:, b * 128:(b + 1) * 128],
                                 rhs=ziT[:, n * 512:(n + 1) * 512],
                                 start=True, stop=True)
                col = b * 2 + n
                nc.scalar.activation(out=h2[b][:, n * 512:(n + 1) * 512],
                                     in_=hp[:], func=AF.Copy,
                                     accum_out=pay2[:, col:col + 1])
                nc.scalar.activation(out=sq_scr[:],
                                     in_=h2[b][:, n * 512:(n + 1) * 512],
                                     func=AF.Square,
                                     accum_out=pay2[:, 4 + col:5 + col])
        ag2_in = dram.tile([128, AG2F], f32, name="ag2_in")
        nc.sync.dma_start(out=ag2_in[:], in_=pay2[:])
        nc.gpsimd.collective_compute(
            "AllGather", ALU.bypass, ins=[ag2_in[:].opt()],
            outs=[ag2_out[:].opt()], replica_groups=[list(range(NCORES))])

        # ---------------- NCT distance loop, second half
        for ic in range(NI_CH // 2, NI_CH):
            for jh in range(2):
                dps = ps_d.tile([128, 1024], f32, tag="dps", name="dps")
                # batch the two K=1 prefills (one LDWEIGHTS), then the two dots
                for jq in range(2):
                    off = jh * 1024 + jq * 512
                    sl = slice(jq * 512, (jq + 1) * 512)
                    nc.tensor.matmul(out=dps[:, sl], lhsT=ones_row[:],
                                     rhs=nsq_row[:, off:off + 512],
                                     start=True, stop=False)
                for jq in range(2):
                    off = jh * 1024 + jq * 512
                    sl = slice(jq * 512, (jq + 1) * 512)
                    nc.tensor.matmul(out=dps[:, sl],
                                     lhsT=zpm2[:, ic * 128:(ic + 1) * 128],
                                     rhs=zsT[:, off:off + 512],
                                     start=False, stop=True)
                col = NADD + ic * 2 + jh
                nc.vector.tensor_reduce(out=pay3[:, col:col + 1], in_=dps[:],
                                        axis=AX.X, op=ALU.min)

        # ---------------- glo branch -> mse
        glo_s, glo_b = stats_from_gram(gz_g, gW1T_bf, gW1nat, g_gam, g_bet,
                                       LAT, NS, "glo")
        h_glo = [sb.tile([128, SH_NS], bf16, tag=f"h_glo{b}", name=f"h_glo{b}")
                 for b in range(2)]
        for b in range(2):
            for n in range(SH_NS // 512):
                hp = ps_sm.tile([128, 512], f32, tag="sm", name="hgmm")
                nc.tensor.matmul(out=hp[:], lhsT=gW1T_bf[:, b * 128:(b + 1) * 128],
                                 rhs=t_zT[:, n * 512:(n + 1) * 512],
                                 start=True, stop=True)
                nc.scalar.activation(out=h_glo[b][:, n * 512:(n + 1) * 512],
                                     in_=hp[:], func=AF.Lrelu,
                                     bias=glo_b[b][:], scale=glo_s[b][:],
                                     alpha=LRELU)
        dtile = sb.tile([SIZE, SH_NS], f32, name="dtile")
        for n in range(SH_NS // 512):
            xp = ps_sm.tile([SIZE, 512], f32, tag="sm", name="xgmm")
            for b in range(2):
                nc.tensor.matmul(out=xp[:], lhsT=gW2T_bf[b][:],
                                 rhs=h_glo[b][:, n * 512:(n + 1) * 512],
                                 start=(b == 0), stop=(b == 1))
            nc.vector.scalar_tensor_tensor(
                out=dtile[:, n * 512:(n + 1) * 512], in0=xp[:], scalar=g_b2[:],
                in1=t_xT[:, n * 512:(n + 1) * 512], op0=ALU.add, op1=ALU.subtract)
        msesq = sb.tile([SIZE, SH_NS], bf16, tag="sq64", name="msesq")
        nc.scalar.activation(out=msesq[:], in_=dtile[:], func=AF.Square,
                             accum_out=pay3[:SIZE, 65:66])

        # ---------------- AG2 combine -> X_ind -> S partials
        ag2l = sb.tile([128, NCORES, AG2F], f32, name="ag2l")
        nc.sync.dma_start(out=ag2l[:],
                          in_=ag2_out[:].rearrange("(c p) f -> p c f", p=128))
        sums2 = sb.tile([128, AG2F], f32, name="sums2")
        nc.vector.tensor_tensor(out=sums2[:], in0=ag2l[:, 0, :],
                                in1=ag2l[:, 1, :], op=ALU.add)
        for c in range(2, NCORES):
            nc.vector.tensor_tensor(out=sums2[:], in0=sums2[:],
                                    in1=ag2l[:, c, :], op=ALU.add)
        h2_s, h2_b = [], []
        for b in range(2):
            tot = sb.tile([128, 1], f32, tag=f"h2tot{b}", name=f"h2tot{b}")
            nc.vector.tensor_tensor(out=tot[:], in0=sums2[:, 2 * b:2 * b + 1],
                                    in1=sums2[:, 2 * b + 1:2 * b + 2], op=ALU.add)
            mu = sb.tile([128, 1], f32, tag=f"h2mu{b}", name=f"h2mu{b}")
            nc.scalar.activation(out=mu[:], in_=tot[:], func=AF.Copy,
                                 scale=1.0 / NIND)
            tot2 = sb.tile([128, 1], f32, tag=f"h2tot2{b}", name=f"h2tot2{b}")
            nc.vector.tensor_tensor(out=tot2[:], in0=sums2[:, 4 + 2 * b:5 + 2 * b],
                                    in1=sums2[:, 5 + 2 * b:6 + 2 * b], op=ALU.add)
            s, bb_ = _stat_tail(tot2, mu, g_gam[b], g_bet[b], NIND, f"h2{b}")
            h2_s.append(s)
            h2_b.append(bb_)
        h2a = [sb.tile([128, SH_NI], f32r, tag=f"h2a{b}", name=f"h2a{b}")
               for b in range(2)]
        for b in range(2):
            nc.scalar.activation(out=h2a[b][:], in_=h2[b][:], func=AF.Lrelu,
                                 bias=h2_b[b][:], scale=h2_s[b][:], alpha=LRELU)
        xiT = sb.tile([SIZE, SH_NI], f32r, name="xiT")
        for n in range(SH_NI // 512):
            xp = ps_sm.tile([SIZE, 512], f32, tag="sm", name="ximm")
            for b in range(2):
                nc.tensor.matmul(out=xp[:], lhsT=gW2T_32[b][:],
                                 rhs=h2a[b][:, n * 512:(n + 1) * 512],
                                 start=(b == 0), stop=(b == 1))
            nc.vector.tensor_scalar_add(out=xiT[:, n * 512:(n + 1) * 512],
                                        in0=xp[:], scalar1=g_b2[:])
        xin = sb.tile([128, SH_NI // 128, SIZE], f32r, name="xin")
        for g in range(SH_NI // 128):
            tp = ps_sm.tile([128, SIZE], f32r, tag="sm", name="xi_tp")
            nc.tensor.transpose(out=tp[:], in_=xiT[:, g * 128:(g + 1) * 128],
                                identity=identr[:SIZE, :SIZE])
            nc.scalar.copy(out=xin[:, g, :], in_=tp[:])
        praw = ps_acc.tile([SIZE, SIZE], f32, tag="acc", name="praw")
        for g in range(SH_NI // 128):
            nc.tensor.matmul(out=praw[:], lhsT=xin[:, g, :], rhs=xin[:, g, :],
                             start=(g == 0), stop=(g == SH_NI // 128 - 1))
        nc.scalar.copy(out=pay3[:SIZE, 0:SIZE], in_=praw[:])
        nc.vector.reduce_sum(out=pay3[:SIZE, SIZE:SIZE + 1], in_=xiT[:], axis=AX.X)

        # ---------------- AG3 + combine
        ag3_in = dram.tile([128, AG3F], f32, name="ag3_in")
        nc.sync.dma_start(out=ag3_in[:], in_=pay3[:])
        nc.gpsimd.collective_compute(
            "AllGather", ALU.bypass, ins=[ag3_in[:].opt()],
            outs=[ag3_out[:].opt()], replica_groups=[list(range(NCORES))])
        ag3l = sb.tile([128, NCORES, AG3F], f32, name="ag3l")
        nc.sync.dma_start(out=ag3l[:],
                          in_=ag3_out[:].rearrange("(c p) f -> p c f", p=128))
        sum3 = sb.tile([128, NADD], f32, name="sum3")
        nc.vector.tensor_tensor(out=sum3[:], in0=ag3l[:, 0, 0:NADD],
                                in1=ag3l[:, 1, 0:NADD], op=ALU.add)
        for c in range(2, NCORES):
            nc.vector.tensor_tensor(out=sum3[:], in0=sum3[:],
                                    in1=ag3l[:, c, 0:NADD], op=ALU.add)
        dmin = sb.tile([128, 32], f32, name="dmin")
        nc.vector.tensor_tensor(out=dmin[:], in0=ag3l[:, 0, NADD:AG3F],
                                in1=ag3l[:, 1, NADD:AG3F], op=ALU.min)
        for c in range(2, NCORES):
            nc.vector.tensor_tensor(out=dmin[:], in0=dmin[:],
                                    in1=ag3l[:, c, NADD:AG3F], op=ALU.min)
        dmin16 = sb.tile([128, 16], f32, name="dmin16")
        dmv = dmin[:].rearrange("p (i h) -> p i h", h=2)
        nc.vector.tensor_tensor(out=dmin16[:], in0=dmv[:, :, 0], in1=dmv[:, :, 1],
                                op=ALU.min)
        dsum = sb.tile([128, 1], f32, name="dsum")
        nc.vector.reduce_sum(out=dsum[:], in_=dmin16[:], axis=AX.X)

        # ---------------- final assembly (fp32 [64,64])

        S64 = SIZE

        def new64(tag):
            return sb.tile([S64, S64], f32, tag=tag, name=tag)

        fin64 = sb.tile([S64, 8], f32, name="fin64")
        C_t = new64("C_t")
        nc.vector.tensor_tensor(out=C_t[:], in0=Lc[:], in1=LTc[:], op=ALU.subtract)
        nc.scalar.activation(out=C_t[:], in_=C_t[:], func=AF.Sigmoid)
        nc.vector.tensor_tensor(out=C_t[:], in0=C_t[:], in1=offd[:], op=ALU.mult)
        CT_t = new64("CT_t")
        nc.vector.tensor_tensor(out=CT_t[:], in0=LTc[:], in1=Lc[:], op=ALU.subtract)
        nc.scalar.activation(out=CT_t[:], in_=CT_t[:], func=AF.Sigmoid)
        nc.vector.tensor_tensor(out=CT_t[:], in0=CT_t[:], in1=offd[:], op=ALU.mult)
        U_t = new64("U_t")
        nc.vector.tensor_tensor(out=U_t[:], in0=CT_t[:], in1=C_t[:], op=ALU.add)
        cc_ps = ps_sm.tile([S64, S64], f32, tag="sm", name="cc_ps")
        nc.tensor.matmul(out=cc_ps[:], lhsT=CT_t[:], rhs=C_t[:],
                         start=True, stop=True)
        lt_t = new64("lt_t")
        nc.vector.tensor_tensor(out=lt_t[:], in0=cc_ps[:], in1=CT_t[:], op=ALU.mult)
        nc.vector.reduce_sum(out=fin64[:, 0:1], in_=lt_t[:], axis=AX.X)

        csum = sb.tile([S64, 1], f32, name="csum")
        nc.vector.tensor_copy(out=csum[:], in_=sum3[:S64, S64:S64 + 1])
        cr_ps = ps_sm.tile([1, S64], f32, tag="sm", name="cr_ps")
        nc.tensor.transpose(out=cr_ps[:], in_=csum[:], identity=ident_32[:S64, :S64])
        csr = sb.tile([1, S64], f32, name="csr")
        nc.scalar.copy(out=csr[:], in_=cr_ps[:])
        mr = sb.tile([1, S64], f32, name="mr")
        nc.scalar.activation(out=mr[:], in_=csr[:], func=AF.Copy, scale=1.0 / NIND)
        outer_ps = ps_sm.tile([S64, S64], f32, tag="sm", name="outer_ps")
        nc.tensor.matmul(out=outer_ps[:], lhsT=mr[:], rhs=csr[:],
                         start=True, stop=True)
        S_t = new64("S_t")
        nc.vector.tensor_tensor(out=S_t[:], in0=sum3[:S64, 0:S64], in1=outer_ps[:],
                                op=ALU.subtract)
        dtmp = new64("dtmp")
        nc.vector.tensor_tensor(out=dtmp[:], in0=S_t[:], in1=eye[:], op=ALU.mult)
        s2 = sb.tile([S64, 1], f32, name="s2")
        nc.vector.reduce_sum(out=s2[:], in_=dtmp[:], axis=AX.X)
        r2 = sb.tile([S64, 1], f32, name="r2")
        nc.vector.reciprocal(out=r2[:], in_=s2[:])
        s2r_ps = ps_sm.tile([1, S64], f32, tag="sm", name="s2r_ps")
        nc.tensor.transpose(out=s2r_ps[:], in_=s2[:], identity=ident_32[:S64, :S64])
        s2row = sb.tile([1, S64], f32, name="s2row")
        nc.scalar.copy(out=s2row[:], in_=s2r_ps[:])
        onesr64 = sb.tile([1, S64], f32, tag="onesr64", name="onesr64")
        nc.vector.memset(onesr64[:], 1.0)
        s2b_ps = ps_sm.tile([S64, S64], f32, tag="sm", name="s2b_ps")
        nc.tensor.matmul(out=s2b_ps[:], lhsT=onesr64[:], rhs=s2row[:],
                         start=True, stop=True)
        s2b = new64("s2b")
        nc.scalar.copy(out=s2b[:], in_=s2b_ps[:])
        SS = new64("SS")
        nc.vector.tensor_tensor(out=SS[:], in0=S_t[:], in1=S_t[:], op=ALU.mult)
        F_t = new64("F_t")
        nc.vector.tensor_scalar_mul(out=F_t[:], in0=SS[:], scalar1=r2[:])
        dg = new64("dg")
        nc.vector.tensor_tensor(out=dg[:], in0=s2b[:], in1=F_t[:], op=ALU.subtract)
        nc.vector.tensor_tensor(out=dg[:], in0=dg[:], in1=eye[:], op=ALU.add)
        B_t = new64("B_t")
        nc.vector.reciprocal(out=B_t[:], in_=dg[:])
        nc.vector.tensor_tensor(out=B_t[:], in0=B_t[:], in1=offd[:], op=ALU.mult)
        P_t = new64("P_t")
        nc.vector.tensor_tensor(out=P_t[:], in0=U_t[:], in1=B_t[:], op=ALU.mult)
        Q_t = new64("Q_t")
        nc.vector.tensor_tensor(out=Q_t[:], in0=C_t[:], in1=B_t[:], op=ALU.mult)
        ptq_ps = ps_sm.tile([S64, S64], f32, tag="sm", name="ptq_ps")
        nc.tensor.matmul(out=ptq_ps[:], lhsT=P_t[:], rhs=Q_t[:],
                         start=True, stop=True)
        t1_t = new64("t1_t")
        nc.vector.tensor_tensor(out=t1_t[:], in0=SS[:], in1=ptq_ps[:], op=ALU.mult)
        nc.vector.reduce_sum(out=fin64[:, 1:2], in_=t1_t[:], axis=AX.X)
        A_t = new64("A_t")
        nc.vector.tensor_tensor(out=A_t[:], in0=P_t[:], in1=S_t[:], op=ALU.mult)
        Bt_t = new64("Bt_t")
        nc.vector.tensor_tensor(out=Bt_t[:], in0=Q_t[:], in1=S_t[:], op=ALU.mult)
        nc.vector.tensor_scalar_mul(out=Bt_t[:], in0=Bt_t[:], scalar1=r2[:])
        ab_ps = ps_sm.tile([S64, S64], f32, tag="sm", name="ab_ps")
        nc.tensor.matmul(out=ab_ps[:], lhsT=A_t[:], rhs=Bt_t[:],
                         start=True, stop=True)
        t2_t = new64("t2_t")
        nc.vector.tensor_tensor(out=t2_t[:], in0=S_t[:], in1=ab_ps[:], op=ALU.mult)
        nc.vector.reduce_sum(out=fin64[:, 2:3], in_=t2_t[:], axis=AX.X)
        g1 = new64("t1_t")
        nc.vector.tensor_tensor(out=g1[:], in0=P_t[:], in1=SS[:], op=ALU.mult)
        gc = sb.tile([S64, 1], f32, tag="gcol", name="gcol")
        nc.vector.reduce_sum(out=gc[:], in_=g1[:], axis=AX.X)
        d1 = new64("t2_t")
        nc.vector.tensor_tensor(out=d1[:], in0=Q_t[:], in1=SS[:], op=ALU.mult)
        dc = sb.tile([S64, 1], f32, tag="dcol", name="dcol")
        nc.vector.reduce_sum(out=dc[:], in_=d1[:], axis=AX.X)
        t3c = sb.tile([S64, 1], f32, tag="t3col", name="t3col")
        nc.vector.tensor_tensor(out=t3c[:], in0=gc[:], in1=dc[:], op=ALU.mult)
        nc.vector.tensor_tensor(out=t3c[:], in0=t3c[:], in1=r2[:], op=ALU.mult)
        nc.vector.tensor_tensor(out=t3c[:], in0=t3c[:], in1=r2[:], op=ALU.mult)
        nc.vector.tensor_copy(out=fin64[:, 3:4], in_=t3c[:])
        t4_t = new64("lt_t")
        nc.vector.tensor_tensor(out=t4_t[:], in0=U_t[:], in1=C_t[:], op=ALU.mult)
        nc.vector.reduce_sum(out=fin64[:, 4:5], in_=t4_t[:], axis=AX.X)
        r2b = new64("dtmp")
        nc.vector.reciprocal(out=r2b[:], in_=s2b[:])
        ss_t = new64("t1_t")
        nc.vector.tensor_tensor(out=ss_t[:], in0=F_t[:], in1=r2b[:], op=ALU.mult)
        nc.vector.tensor_tensor(out=ss_t[:], in0=ss_t[:], in1=offd[:], op=ALU.mult)
        nc.vector.reduce_sum(out=fin64[:, 5:6], in_=ss_t[:], axis=AX.X)
        nc.vector.tensor_copy(out=fin64[:, 6:7], in_=sum3[:S64, 65:66])
        nc.vector.memset(fin64[:, 7:8], 0.0)

        f64_ps = ps_sm.tile([1, 8], f32, tag="sm", name="f64_ps")
        nc.tensor.matmul(out=f64_ps[:], lhsT=ones64[:], rhs=fin64[:],
                         start=True, stop=True)
        frow = sb.tile([1, 8], f32, name="frow")
        nc.scalar.copy(out=frow[:], in_=f64_ps[:])
        fin128 = sb.tile([128, 2], f32, name="fin128")
        nc.vector.tensor_copy(out=fin128[:, 0:1], in_=dsum[:])
        nc.vector.tensor_copy(out=fin128[:, 1:2], in_=zpsq_col[:])
        f128_ps = ps_sm.tile([1, 2], f32, tag="sm", name="f128_ps")
        nc.tensor.matmul(out=f128_ps[:], lhsT=ones128[:], rhs=fin128[:],
                         start=True, stop=True)
        grow = sb.tile([1, 2], f32, name="grow")
        nc.scalar.copy(out=grow[:], in_=f128_ps[:])

        acc = sb.tile([1, 1], f32, name="acc_sc")
        tmp = sb.tile([1, 1], f32, tag="tmp_sc", name="tmp_sc")
        nc.vector.tensor_copy(out=acc[:], in_=frow[:, 0:1])
        nc.scalar.activation(out=tmp[:], in_=frow[:, 6:7], func=AF.Copy,
                             scale=1.0 / (NS * SIZE))
        nc.vector.tensor_tensor(out=acc[:], in0=acc[:], in1=tmp[:], op=ALU.add)
        nc.scalar.activation(out=tmp[:], in_=grow[:, 0:1], func=AF.Copy,
                             scale=1.0 / (BTR * LAT))
        nc.vector.tensor_tensor(out=acc[:], in0=acc[:], in1=tmp[:], op=ALU.add)
        nc.scalar.activation(out=tmp[:], in_=grow[:, 1:2], func=AF.Copy,
                             scale=0.25 / (BTR * LAT))
        nc.vector.tensor_tensor(out=acc[:], in0=acc[:], in1=tmp[:], op=ALU.add)
        nc.vector.tensor_tensor(out=acc[:], in0=acc[:], in1=frow[:, 1:2],
                                op=ALU.add)
        nc.scalar.activation(out=tmp[:], in_=frow[:, 2:3], func=AF.Copy,
                             scale=-2.0)
        nc.vector.tensor_tensor(out=acc[:], in0=acc[:], in1=tmp[:], op=ALU.add)
        nc.vector.tensor_tensor(out=acc[:], in0=acc[:], in1=frow[:, 3:4],
                                op=ALU.add)
        nc.vector.tensor_tensor(out=acc[:], in0=acc[:], in1=frow[:, 4:5],
                                op=ALU.subtract)
        nc.scalar.activation(out=tmp[:], in_=frow[:, 5:6], func=AF.Copy,
                             scale=float(S64 - 2))
        nc.vector.tensor_tensor(out=acc[:], in0=acc[:], in1=tmp[:], op=ALU.add)
        nc.sync.dma_start(out=out_d[:], in_=acc[:])

    _split_multi_waits(nc)
    return nc


def _stage_inputs(I):
    g = lambda k: np.asarray(I[k], dtype=np.float32)
    z = g("z_logits")
    X = g("X")
    ntr = g("noise_trans")
    nind = g("noise_indep")
    perm = np.asarray(I["perm_idx"], dtype=np.int32).reshape(-1)
    L = g("conn_logits")

    def bf(a):
        return np.ascontiguousarray(a.astype(bfnp))

    def f(a):
        return np.ascontiguousarray(a.astype(np.float32))

    z_e32 = np.concatenate([z, np.ones((NS, 1), np.float32)], axis=1)

    cbf_blob = np.zeros((128, CBF_W), bfnp)
    c32_blob = np.zeros((128, C32_W), np.float32)
    cfr_blob = np.zeros((128, CFR_W), np.float32)

    def put(blob, m, name, arr):
        r, c0, w = m[name]
        blob[:r, c0:c0 + w] = arr.astype(blob.dtype)

    put(cbf_blob, CBF_MAP, "ident_bf", np.eye(128, dtype=np.float32))
    put(cbf_blob, CBF_MAP, "gW1T_bf", g("glo_W1").T)
    put(cbf_blob, CBF_MAP, "gW1nat0", g("glo_W1")[:128])
    put(cbf_blob, CBF_MAP, "gW1nat1", g("glo_W1")[128:])
    put(cbf_blob, CBF_MAP, "gW2T_bf0", g("glo_W2").T[:128])
    put(cbf_blob, CBF_MAP, "gW2T_bf1", g("glo_W2").T[128:])
    put(cbf_blob, CBF_MAP, "tW1T_bf", g("tr_W1").T)
    put(cbf_blob, CBF_MAP, "tW1nat0", g("tr_W1")[:128])
    put(cbf_blob, CBF_MAP, "tW1nat1", g("tr_W1")[128:])
    put(cbf_blob, CBF_MAP, "tW2T_bf0", g("tr_W2").T[:128])
    put(cbf_blob, CBF_MAP, "tW2T_bf1", g("tr_W2").T[128:])
    put(cbf_blob, CBF_MAP, "ones_row", np.ones((1, 128), np.float32))
    put(cbf_blob, CBF_MAP, "ones_col", np.ones((128, 1), np.float32))
    put(c32_blob, C32_MAP, "ident_32", np.eye(128, dtype=np.float32))
    put(c32_blob, C32_MAP, "eye", np.eye(SIZE, dtype=np.float32))
    put(c32_blob, C32_MAP, "offd", 1.0 - np.eye(SIZE, dtype=np.float32))
    put(c32_blob, C32_MAP, "L", L)
    put(c32_blob, C32_MAP, "LT", L.T)
    put(c32_blob, C32_MAP, "g_gam0", g("glo_gamma")[:128].reshape(-1, 1))
    put(c32_blob, C32_MAP, "g_gam1", g("glo_gamma")[128:].reshape(-1, 1))
    put(c32_blob, C32_MAP, "g_bet0", g("glo_beta")[:128].reshape(-1, 1))
    put(c32_blob, C32_MAP, "g_bet1", g("glo_beta")[128:].reshape(-1, 1))
    put(c32_blob, C32_MAP, "t_gam0", g("tr_gamma")[:128].reshape(-1, 1))
    put(c32_blob, C32_MAP, "t_gam1", g("tr_gamma")[128:].reshape(-1, 1))
    put(c32_blob, C32_MAP, "t_bet0", g("tr_beta")[:128].reshape(-1, 1))
    put(c32_blob, C32_MAP, "t_bet1", g("tr_beta")[128:].reshape(-1, 1))
    put(c32_blob, C32_MAP, "g_b2", g("glo_b2").reshape(-1, 1))
    put(c32_blob, C32_MAP, "t_b2", g("tr_b2").reshape(-1, 1))
    put(c32_blob, C32_MAP, "ones64", np.ones((SIZE, 1), np.float32))
    put(c32_blob, C32_MAP, "ones128", np.ones((128, 1), np.float32))
    put(cfr_blob, CFR_MAP, "identr", np.eye(128, dtype=np.float32))
    put(cfr_blob, CFR_MAP, "gW2T_320", g("glo_W2").T[:128])
    put(cfr_blob, CFR_MAP, "gW2T_321", g("glo_W2").T[128:])
    put(cfr_blob, CFR_MAP, "gW1T_32", g("glo_W1").T)
    put(cfr_blob, CFR_MAP, "tW1T_32", g("tr_W1").T)
    put(cfr_blob, CFR_MAP, "tW2T_320", g("tr_W2").T[:128])
    put(cfr_blob, CFR_MAP, "tW2T_321", g("tr_W2").T[128:])

    shared = {
        "zext": bf(z_e32),
        "ntrT": bf(ntr.T),
        "ntr_ext": bf(np.concatenate([ntr, np.ones((BTR, 1), np.float32)], 1)),
        "cbf": cbf_blob, "c32": c32_blob, "cfr": cfr_blob,
    }
    zT = z.T
    XT = X.T
    nindT = nind.T
    maps = []
    for c in range(NCORES):
        m = dict(shared)
        m["znat32"] = f(z_e32[c * SH_NS:(c + 1) * SH_NS, :])
        m["nind_nat"] = f(np.concatenate(
            [nind[c * SH_NI:(c + 1) * SH_NI],
             np.ones((SH_NI, 1), np.float32)], 1))
        m["zT_sh"] = bf(zT[:, c * SH_NS:(c + 1) * SH_NS])
        m["xT_sh"] = bf(XT[:, c * SH_NS:(c + 1) * SH_NS])
        m["nindT32"] = f(nindT[:, c * SH_NI:(c + 1) * SH_NI])
        m["perm_sh"] = np.ascontiguousarray(
            perm[c * SH_J:(c + 1) * SH_J].reshape(NI_CH, 128).T)
        maps.append(m)
    return maps


def _get_nc():
    if "nc" not in _CACHE:
        _install_profshim()
        _CACHE["nc"] = _build_program()
    return _CACHE["nc"]


def run(inputs, trace=False):
    nc = _get_nc()
    maps = _stage_inputs(inputs)
    res = run_bass_kernel_spmd(nc, maps, list(range(NCORES)), trace=trace)
    val = np.float32(res.results[0]["out"].reshape(-1)[0])
    return val, res


def kernel(**inputs) -> np.ndarray:
    val, _ = run(inputs, trace=False)
    return np.asarray(val, dtype=np.float32)


if __name__ == "__main__":
    nc = _get_nc()
    ninst = sum(len(bb.instructions) for bb in nc.main_func.blocks)
    print("built ok, instructions:", ninst)



# revision 15
# speedup vs baseline: 1.2831x; 1.2831x over previous
"""Trainium2 Bass kernel for nn_CausalityChainModel (loss_fn), 8-core SPMD.

Self-contained: takes FULL inputs, shards internally across 8 NeuronCores,
runs one Bass/Tile program via run_bass_kernel_spmd, returns the scalar loss.

v2 architecture (validated numerically on CPU):
- loss_nct's min over Zs = z_logits[perm] rows is permutation-invariant, so
  the indirect gather is gone; each core min-reduces over its own z shard.
- NCT distances via fp8e4 DoubleRow matmuls (0.5 cyc/row) with |z_j|^2
  folded in as a 65th contraction row (lhsT ones / rhs nsq), killing the
  K=1 prefill entirely. Min via vector tensor_tensor_reduce (fused pairwise
  min + reduce = one DVE pass per [128,1024] pair). loss_nct ~ 2e-5 of the
  total, so fp8 error is irrelevant.
- noise_indep gram replicated per-core in fp16 (6.5us PE) instead of
  collected -> only TWO chained collectives:
    AGa: z-gram partials (glo BN stats) + h2 moment partials
    AGb: S-gram partials + colsum + mse + per-i NCT mins
  plus a dummy 1-col AllGather at t~0 to absorb the CC cold-start.
- glo recon (mse, ~3e-5 of total) in fp8 DoubleRow; X_ind chain in f32r.
- C / loss_trans math right at the start (sigmoid table load off the tail).
"""
import os
import sys
import types
import contextlib

for _p in ("/opt/trn_rl_repo", "/root/.axon_site"):
    if _p not in sys.path:
        sys.path.insert(0, _p)

import numpy as np
import ml_dtypes

import concourse.bass as bass
import concourse.tile as tile
from concourse import mybir
from concourse.bass_utils import run_bass_kernel_spmd

SIZE, NS, LAT, NOISE, HID, BTR, NIND = 64, 16384, 128, 64, 256, 2048, 8192
NCORES = 8
SH_NS = NS // NCORES      # 2048
SH_NI = NIND // NCORES    # 1024
BN_EPS = 1e-5
LRELU = 0.01

f32 = mybir.dt.float32
f32r = mybir.dt.float32r
bf16 = mybir.dt.bfloat16
fp16 = mybir.dt.float16
fp8 = mybir.dt.float8e4
i32 = mybir.dt.int32
AF = mybir.ActivationFunctionType
ALU = mybir.AluOpType
AX = mybir.AxisListType
DR = mybir.MatmulPerfMode.DoubleRow
bfnp = ml_dtypes.bfloat16
f8np = ml_dtypes.float8_e4m3

NICH = BTR // 128         # 16 Zp chunks
BIGF = 3.0e38

AGAF = (LAT + 1) + 8      # z-gram 129 | h2 stat cols 8
AGBF = SIZE + 2 + NICH    # S 64 | colsum 1 | mse 1 | dmin 16
NADD = SIZE + 2           # cols of AGb combined with add

# fp8 constant blob [128, C8W]: DoubleRow weight layouts, viewed [p, 2, M]
C8_MAP = {
    "gW1T8": (64, 0, 512),     # [64,2,256] h_glo lhsT (K=128 lat)
    "gW2T8": (128, 512, 128),  # [128,2,64] dtile lhsT (K=256 hid)
    "tW1T8": (32, 640, 512),   # [32,2,256] h_tr lhsT (K=64 noise)
    "tW2T8": (128, 1152, 256),  # [128,2,128] zpm2 lhsT (K=256 hid)
}
C8W = 1408
C32_MAP = {
    "ident_32": (128, 0, 128), "eye": (64, 128, 64), "offd": (64, 192, 64),
    "L": (64, 256, 64), "LT": (64, 320, 64),
    "g_gam": (128, 384, 2), "g_bet": (128, 386, 2),
    "t_gam": (128, 388, 2), "t_bet": (128, 390, 2),
    "g_b2": (64, 392, 1), "t_b2": (128, 393, 1), "tb2m2": (64, 394, 2),
    "ones64": (64, 396, 1), "ones128": (128, 397, 1),
    "tW1T_32": (64, 398, 256), "gW1T_32": (128, 654, 256),
    "tW1nat": (128, 910, 128), "gW1nat": (128, 1038, 256),
}
C32W = 1294
CFR_MAP = {
    "identr": (128, 0, 128),
    "tW1T_r": (64, 128, 256), "tW2T_r": (128, 384, 256),
    "gW1T_r": (128, 640, 256), "gW2T_r": (128, 896, 128),
}
CFRW = 1024

_CACHE = {}


def _install_profshim():
    if "antenv.axon_hooks" in sys.modules:
        return
    try:
        import antenv
        mod = types.ModuleType("antenv.axon_hooks")
        mod._hook = None
        mod.set_axon_ntff_profile_hook = lambda h: setattr(mod, "_hook", h)
        mod.get_axon_ntff_profile_hook = lambda: mod._hook
        sys.modules["antenv.axon_hooks"] = mod
        antenv.axon_hooks = mod
        from trn_agent_boot import trn_boot
        so = "/opt/axon/libaxon_pjrt.so"
        if os.path.exists(so):
            mod.set_axon_ntff_profile_hook(trn_boot._ntff_profile_via_ctypes(so))
        import concourse.bass_utils as bu
        bu.upload_artifacts = lambda tmpdir: str(tmpdir)
    except Exception:
        pass


def _split_multi_waits(nc, max_waits=1):
    """This walrus build rejects >1 sem-wait per instruction: move extras onto
    EventSemaphore nops (cheap, non-pipeline-flushing) placed just before."""
    for bb in nc.main_func.blocks:
        new_insts = []
        for inst in bb.instructions:
            si = inst.sync_info
            if si is not None and len(si.on_wait) > max_waits:
                waits = list(si.on_wait)
                extra, keep = waits[:-max_waits], waits[-max_waits:]
                for i in range(0, len(extra), max_waits):
                    d = mybir.InstEventSemaphore(
                        name=f"{inst.name}-wsplit{i}", ins=[], outs=[])
                    d.engine = inst.engine
                    d.sync_info = mybir.SyncInfo(
                        on_wait=list(extra[i:i + max_waits]), on_update=[])
                    new_insts.append(d)
                inst.sync_info = mybir.SyncInfo(
                    on_wait=list(keep), on_update=list(si.on_update))
            new_insts.append(inst)
        try:
            bb.instructions[:] = new_insts
        except TypeError:
            bb.instructions = new_insts


def _build_program():
    nc = bass.Bass()

    def din(name, shape, dt):
        return nc.dram_tensor(name, shape, dt, kind="ExternalInput")

    znat16_d = din("znat16", [SH_NS, LAT + 1], fp16)   # z shard | ones (gram)
    z8_d = din("z8d", [65, 2, SH_NS], fp8)             # z shard, DR layout
    zprow_d = din("zprow", [1, 2 * BTR], fp8)          # zp8 row 64: ones|zeros
    xT_d = din("xT_sh", [SIZE, SH_NS], bf16)
    ntr16_d = din("ntr16", [BTR, NOISE + 1], fp16)     # noise_trans | ones
    ntrT8_d = din("ntrT8", [32, 2, BTR], fp8)          # h_tr rhs, DR layout
    nind16_d = din("nind16", [NIND, NOISE + 1], fp16)  # FULL noise_indep|ones
    nindT32_d = din("nindT32", [NOISE, SH_NI], f32r)   # ind chain rhs shard
    c8_d = din("c8", [128, C8W], fp8)
    c32_d = din("c32", [128, C32W], f32)
    cfr_d = din("cfr", [128, CFRW], f32r)

    out_d = nc.dram_tensor("out", [1, 1], f32, kind="ExternalOutput")

    agw_out = nc.dram_tensor("agw_out", [NCORES, 1], f32, addr_space="Shared")
    aga_out = nc.dram_tensor("aga_out", [NCORES * 128, AGAF], f32,
                             addr_space="Shared")
    agb_out = nc.dram_tensor("agb_out", [NCORES * 128, AGBF], f32,
                             addr_space="Shared")

    with tile.TileContext(nc) as tc, contextlib.ExitStack() as ctx:
        const = ctx.enter_context(tc.tile_pool(name="const", bufs=1))
        sb = ctx.enter_context(tc.tile_pool(name="sb", bufs=1))
        sb3 = ctx.enter_context(tc.tile_pool(name="sb3", bufs=3))
        ps_acc = ctx.enter_context(tc.tile_pool(name="ps_acc", bufs=2,
                                                space="PSUM"))
        ps_sm = ctx.enter_context(tc.tile_pool(name="ps_sm", bufs=2,
                                               space="PSUM"))
        ps_d = ctx.enter_context(tc.tile_pool(name="ps_d", bufs=2,
                                              space="PSUM"))
        dram = ctx.enter_context(tc.tile_pool(name="dram", bufs=1,
                                              space="DRAM"))

        # ---------------- dummy collective: absorb CC cold-start at t~0
        wz = sb.tile([1, 1], f32, name="wz")
        nc.vector.memset(wz[:], 0.0)
        agw_in = dram.tile([1, 1], f32, name="agw_in")
        nc.sync.dma_start(out=agw_in[:], in_=wz[:])
        nc.gpsimd.collective_compute(
            "AllGather", ALU.bypass, ins=[agw_in[:].opt()],
            outs=[agw_out[:].opt()], replica_groups=[list(range(NCORES))])

        # ---------------- constants + inputs (gram feeders first)
        c32 = const.tile([128, C32W], f32, name="c32")
        nc.sync.dma_start(out=c32[:], in_=c32_d[:])
        c8 = const.tile([128, C8W], fp8, name="c8")
        nc.sync.dma_start(out=c8[:], in_=c8_d[:])
        cfr = const.tile([128, CFRW], f32r, name="cfr")
        nc.sync.dma_start(out=cfr[:], in_=cfr_d[:])

        t_ntr = sb.tile([128, BTR // 128, NOISE + 1], fp16, name="t_ntr")
        nc.sync.dma_start(out=t_ntr[:],
                          in_=ntr16_d[:].rearrange("(c p) f -> p c f", p=128))
        t_nin = sb.tile([128, NIND // 128, NOISE + 1], fp16, name="t_nin")
        for q in range(4):
            nc.sync.dma_start(
                out=t_nin[:, q * 16:(q + 1) * 16, :],
                in_=nind16_d[q * 2048:(q + 1) * 2048, :]
                .rearrange("(c p) f -> p c f", p=128))
        t_znat = sb.tile([128, SH_NS // 128, LAT + 1], fp16, name="t_znat")
        for q in range(2):
            nc.sync.dma_start(
                out=t_znat[:, q * 8:(q + 1) * 8, :],
                in_=znat16_d[q * 1024:(q + 1) * 1024, :]
                .rearrange("(c p) f -> p c f", p=128))
        t_ntrT8 = sb.tile([32, 2, BTR], fp8, name="t_ntrT8")
        nc.sync.dma_start(out=t_ntrT8[:], in_=ntrT8_d[:])
        t_nindT = sb.tile([NOISE, SH_NI], f32r, name="t_nindT")
        nc.sync.dma_start(out=t_nindT[:], in_=nindT32_d[:])
        # z8: rows 0-63 latents (DR tiles), row 64 = (nsq | zeros)
        z8 = sb.tile([65, 2, SH_NS], fp8, name="z8")
        nc.sync.dma_start(out=z8[:], in_=z8_d[:])
        t_xT = sb.tile([SIZE, SH_NS], bf16, name="t_xT")
        nc.sync.dma_start(out=t_xT[:], in_=xT_d[:])

        def V(blob, m, name):
            r, c0, w = m[name]
            return blob[:r, c0:c0 + w]

        gW1T8 = V(c8, C8_MAP, "gW1T8").rearrange("p (t m) -> p t m", t=2)
        gW2T8 = V(c8, C8_MAP, "gW2T8").rearrange("p (t m) -> p t m", t=2)
        tW1T8 = V(c8, C8_MAP, "tW1T8").rearrange("p (t m) -> p t m", t=2)
        tW2T8 = V(c8, C8_MAP, "tW2T8").rearrange("p (t m) -> p t m", t=2)
        ident_32 = V(c32, C32_MAP, "ident_32")
        eye = V(c32, C32_MAP, "eye")
        offd = V(c32, C32_MAP, "offd")
        Lc = V(c32, C32_MAP, "L")
        LTc = V(c32, C32_MAP, "LT")
        g_gam = V(c32, C32_MAP, "g_gam")
        g_bet = V(c32, C32_MAP, "g_bet")
        t_gam = V(c32, C32_MAP, "t_gam")
        t_bet = V(c32, C32_MAP, "t_bet")
        g_b2 = V(c32, C32_MAP, "g_b2")
        t_b2 = V(c32, C32_MAP, "t_b2")
        tb2m2 = V(c32, C32_MAP, "tb2m2")
        ones64 = V(c32, C32_MAP, "ones64")
        ones128 = V(c32, C32_MAP, "ones128")
        tW1T_32 = V(c32, C32_MAP, "tW1T_32")
        gW1T_32 = V(c32, C32_MAP, "gW1T_32")
        tW1nat = V(c32, C32_MAP, "tW1nat")
        gW1nat = V(c32, C32_MAP, "gW1nat")
        identr = V(cfr, CFR_MAP, "identr")
        tW1T_r = V(cfr, CFR_MAP, "tW1T_r")
        tW2T_r = V(cfr, CFR_MAP, "tW2T_r")
        gW1T_r = V(cfr, CFR_MAP, "gW1T_r")
        gW2T_r = V(cfr, CFR_MAP, "gW2T_r")
        eps_col = const.tile([128, 1], f32, name="eps_col")
        nc.vector.memset(eps_col[:], BN_EPS)

        # ---------------- C block (depends only on conn_logits; runs early)
        fin64 = sb.tile([SIZE, 8], f32, name="fin64")
        C_t = sb.tile([SIZE, SIZE], f32, name="C_t")
        nc.vector.tensor_tensor(out=C_t[:], in0=Lc[:], in1=LTc[:],
                                op=ALU.subtract)
        nc.scalar.activation(out=C_t[:], in_=C_t[:], func=AF.Sigmoid)
        nc.vector.tensor_tensor(out=C_t[:], in0=C_t[:], in1=offd[:],
                                op=ALU.mult)
        CT_t = sb.tile([SIZE, SIZE], f32, name="CT_t")
        nc.vector.tensor_tensor(out=CT_t[:], in0=LTc[:], in1=Lc[:],
                                op=ALU.subtract)
        nc.scalar.activation(out=CT_t[:], in_=CT_t[:], func=AF.Sigmoid)
        nc.vector.tensor_tensor(out=CT_t[:], in0=CT_t[:], in1=offd[:],
                                op=ALU.mult)
        U_t = sb.tile([SIZE, SIZE], f32, name="U_t")
        nc.vector.tensor_tensor(out=U_t[:], in0=CT_t[:], in1=C_t[:],
                                op=ALU.add)
        cc_ps = ps_sm.tile([SIZE, SIZE], f32, tag="sm", name="cc_ps")
        nc.tensor.matmul(out=cc_ps[:], lhsT=CT_t[:], rhs=C_t[:],
                         start=True, stop=True)
        lt_t = sb.tile([SIZE, SIZE], f32, tag="sc64", name="lt_t")
        nc.vector.tensor_tensor(out=lt_t[:], in0=cc_ps[:], in1=CT_t[:],
                                op=ALU.mult)
        nc.vector.reduce_sum(out=fin64[:, 0:1], in_=lt_t[:], axis=AX.X)
        t4_t = sb.tile([SIZE, SIZE], f32, tag="sc64", name="t4_t")
        nc.vector.tensor_tensor(out=t4_t[:], in0=U_t[:], in1=C_t[:],
                                op=ALU.mult)
        nc.vector.reduce_sum(out=fin64[:, 4:5], in_=t4_t[:], axis=AX.X)

        # ---------------- nsq: per-sample |z|^2 as fp8 row 64 of z8
        nsq_col = sb.tile([128, SH_NS // 128], f32, name="nsq_col")
        for k in range(SH_NS // 128):
            sq_scr = sb3.tile([128, LAT], bf16, tag="nrm_scr", name="nrm_scr")
            nc.scalar.activation(out=sq_scr[:], in_=t_znat[:, k, :LAT],
                                 func=AF.Square,
                                 accum_out=nsq_col[:, k:k + 1])
        nsqT_ps = ps_sm.tile([SH_NS // 128, 128], f32, tag="sm", name="nsqT_ps")
        nc.tensor.transpose(out=nsqT_ps[:], in_=nsq_col[:],
                            identity=ident_32[:])
        nsqT = sb.tile([SH_NS // 128, 128], fp8, name="nsqT")
        nc.scalar.copy(out=nsqT[:], in_=nsqT_ps[:])
        nsq_dram = dram.tile([1, SH_NS], fp8, name="nsq_dram")
        nc.sync.dma_start(
            out=nsq_dram[:].rearrange("a (p f) -> (a p) f", p=SH_NS // 128),
            in_=nsqT[:])
        nc.sync.dma_start(out=z8[64:65, 0, :], in_=nsq_dram[:])

        # ---------------- local grams: gtr (fp16), nind (fp16, FULL batch),
        # z shard (fp16, partials -> AGa)
        gtr_ps = ps_acc.tile([NOISE, NOISE + 1], f32, tag="acc", name="gtr_ps")
        for k in range(BTR // 128):
            nc.tensor.matmul(out=gtr_ps[:], lhsT=t_ntr[:, k, :NOISE],
                             rhs=t_ntr[:, k, :],
                             start=(k == 0), stop=(k == BTR // 128 - 1))
        gtr = sb.tile([NOISE, NOISE + 1], f32, name="gtr")
        nc.scalar.copy(out=gtr[:], in_=gtr_ps[:])

        gni_ps = ps_acc.tile([NOISE, NOISE + 1], f32, tag="acc", name="gni_ps")
        for k in range(NIND // 128):
            nc.tensor.matmul(out=gni_ps[:], lhsT=t_nin[:, k, :NOISE],
                             rhs=t_nin[:, k, :],
                             start=(k == 0), stop=(k == NIND // 128 - 1))
        gni = sb.tile([NOISE, NOISE + 1], f32, name="gni")
        nc.scalar.copy(out=gni[:], in_=gni_ps[:])

        paya = sb.tile([128, AGAF], f32, name="paya")
        gz_ps = ps_acc.tile([LAT, LAT + 1], f32, tag="acc", name="gz_ps")
        for k in range(SH_NS // 128):
            nc.tensor.matmul(out=gz_ps[:], lhsT=t_znat[:, k, :LAT],
                             rhs=t_znat[:, k, :],
                             start=(k == 0), stop=(k == SH_NS // 128 - 1))
        nc.scalar.copy(out=paya[:, 0:LAT + 1], in_=gz_ps[:])

        # ---------------- BN stat helpers
        def _stat_tail(tot2_c, mu_c, gam, bet, N, tag, nb=2):
            """[128, nb] stats: returns (scale, bias) each [128, nb]."""
            var = sb.tile([128, nb], f32, tag=f"var_{tag}", name=f"var_{tag}")
            nc.scalar.activation(out=var[:], in_=tot2_c, func=AF.Copy,
                                 scale=1.0 / N)
            musq = sb.tile([128, nb], f32, tag=f"musq_{tag}",
                           name=f"musq_{tag}")
            nc.vector.tensor_tensor(out=musq[:], in0=mu_c, in1=mu_c,
                                    op=ALU.mult)
            nc.vector.tensor_tensor(out=var[:], in0=var[:], in1=musq[:],
                                    op=ALU.subtract)
            std = sb.tile([128, nb], f32, tag=f"std_{tag}", name=f"std_{tag}")
            nc.scalar.activation(out=std[:], in_=var[:], func=AF.Sqrt,
                                 bias=eps_col[:])
            rstd = sb.tile([128, nb], f32, tag=f"rstd_{tag}",
                           name=f"rstd_{tag}")
            nc.vector.reciprocal(out=rstd[:], in_=std[:])
            s = sb.tile([128, nb], f32, tag=f"s_{tag}", name=f"s_{tag}")
            nc.vector.tensor_tensor(out=s[:], in0=gam, in1=rstd[:],
                                    op=ALU.mult)
            bb_ = sb.tile([128, nb], f32, tag=f"b_{tag}", name=f"b_{tag}")
            nc.vector.tensor_tensor(out=bb_[:], in0=mu_c, in1=s[:],
                                    op=ALU.mult)
            nc.vector.tensor_tensor(out=bb_[:], in0=bet, in1=bb_[:],
                                    op=ALU.subtract)
            return s, bb_

        def stats_from_gram(gram, w1T_32, w1nat, gam, bet, n_in, N, tag):
            """gram: SBUF f32 [n_in, n_in+1]. Returns ([128,2] s, [128,2] b)."""
            mus = sb.tile([128, 2], f32, tag=f"mus_{tag}", name=f"mus_{tag}")
            esq = sb.tile([128, 2], f32, tag=f"esq_{tag}", name=f"esq_{tag}")
            for b in range(2):
                mm = ps_sm.tile([128, n_in + 1], f32, tag="sm", name="stat_mm")
                nc.tensor.matmul(out=mm[:],
                                 lhsT=w1T_32[:, b * 128:(b + 1) * 128],
                                 rhs=gram, start=True, stop=True)
                prod = sb3.tile([128, n_in], f32, tag="stat_prod",
                                name="stat_prod")
                nc.vector.tensor_tensor(out=prod[:], in0=mm[:, :n_in],
                                        in1=w1nat[:, b * n_in:(b + 1) * n_in],
                                        op=ALU.mult)
                nc.vector.reduce_sum(out=esq[:, b:b + 1], in_=prod[:],
                                     axis=AX.X)
                nc.scalar.activation(out=mus[:, b:b + 1],
                                     in_=mm[:, n_in:n_in + 1],
                                     func=AF.Copy, scale=1.0 / N)
            return _stat_tail(esq[:], mus[:], gam, bet, N, tag)

        tr_s, tr_b = stats_from_gram(gtr[:], tW1T_32, tW1nat, t_gam, t_bet,
                                     NOISE, BTR, "tr")
        ind_s, ind_b = stats_from_gram(gni[:], tW1T_32, tW1nat, t_gam, t_bet,
                                       NOISE, NIND, "ind")

        # ---------------- tr branch: h_tr (fp8 DR) -> zpm2 -> zp8
        h8tr = sb.tile([128, 2, BTR], fp8, name="h8tr")
        for b in range(2):
            for n in range(BTR // 512):
                hp = ps_sm.tile([128, 512], f32, tag="sm", name="htr_mm")
                nc.tensor.matmul(out=hp[:],
                                 lhsT=tW1T8[:, :, b * 128:(b + 1) * 128],
                                 rhs=t_ntrT8[:, :, n * 512:(n + 1) * 512],
                                 start=True, stop=True, perf_mode=DR)
                nc.scalar.activation(out=h8tr[:, b, n * 512:(n + 1) * 512],
                                     in_=hp[:], func=AF.Lrelu,
                                     bias=tr_b[:, b:b + 1],
                                     scale=tr_s[:, b:b + 1], alpha=LRELU)
        # zp8: [65,2,BTR]; rows 0-63 = -2*Zp split over DR tiles,
        # row 64 tile0 = 1 (nsq pickup), tile1 = 0 (DMA'd constant)
        zp8 = sb.tile([65, 2, BTR], fp8, name="zp8")
        nc.sync.dma_start(out=zp8[64:65, :, :], in_=zprow_d[:])
        for t in range(2):
            for n in range(BTR // 512):
                zpp = ps_sm.tile([64, 512], f32, tag="sm", name="zp_mm")
                nc.tensor.matmul(out=zpp[:],
                                 lhsT=tW2T8[:, :, t * 64:(t + 1) * 64],
                                 rhs=h8tr[:, :, n * 512:(n + 1) * 512],
                                 start=True, stop=True, perf_mode=DR)
                nc.scalar.activation(out=zp8[:64, t, n * 512:(n + 1) * 512],
                                     in_=zpp[:], func=AF.Identity,
                                     bias=tb2m2[:64, t:t + 1], scale=-2.0)

        # ---------------- ind chain (f32r): h_ind -> ziT -> h2 (+moments)
        h_ind = [sb.tile([128, SH_NI], f32r, tag=f"h_ind{b}",
                         name=f"h_ind{b}") for b in range(2)]
        for b in range(2):
            for n in range(SH_NI // 512):
                hp = ps_sm.tile([128, 512], f32, tag="sm", name="hi_mm")
                nc.tensor.matmul(out=hp[:],
                                 lhsT=tW1T_r[:, b * 128:(b + 1) * 128],
                                 rhs=t_nindT[:, n * 512:(n + 1) * 512],
                                 start=True, stop=True)
                nc.scalar.activation(out=h_ind[b][:, n * 512:(n + 1) * 512],
                                     in_=hp[:], func=AF.Lrelu,
                                     bias=ind_b[:, b:b + 1],
                                     scale=ind_s[:, b:b + 1], alpha=LRELU)
        ziT = sb.tile([LAT, SH_NI], f32r, name="ziT")
        for n in range(SH_NI // 512):
            zp = ps_sm.tile([LAT, 512], f32, tag="sm", name="zi_mm")
            for b in range(2):
                nc.tensor.matmul(out=zp[:],
                                 lhsT=tW2T_r[:, b * 128:(b + 1) * 128],
                                 rhs=h_ind[b][:, n * 512:(n + 1) * 512],
                                 start=(b == 0), stop=(b == 1))
            nc.vector.tensor_scalar_add(out=ziT[:, n * 512:(n + 1) * 512],
                                        in0=zp[:], scalar1=t_b2[:])
        h2 = [sb.tile([128, SH_NI], f32r, tag=f"h2_{b}", name=f"h2_{b}")
              for b in range(2)]
        for b in range(2):
            for n in range(SH_NI // 512):
                hp = ps_sm.tile([128, 512], f32, tag="sm", name="h2_mm")
                nc.tensor.matmul(out=hp[:],
                                 lhsT=gW1T_r[:, b * 128:(b + 1) * 128],
                                 rhs=ziT[:, n * 512:(n + 1) * 512],
                                 start=True, stop=True)
                col = LAT + 1 + b * 2 + n
                nc.scalar.activation(out=h2[b][:, n * 512:(n + 1) * 512],
                                     in_=hp[:], func=AF.Copy,
                                     accum_out=paya[:, col:col + 1])
        for b in range(2):
            for n in range(SH_NI // 512):
                col = LAT + 5 + b * 2 + n
                sq3 = sb3.tile([128, 512], bf16, tag="h2sq_scr",
                               name="h2sq_scr")
                nc.scalar.activation(out=sq3[:],
                                     in_=h2[b][:, n * 512:(n + 1) * 512],
                                     func=AF.Square,
                                     accum_out=paya[:, col:col + 1])
        # |Zp|^2 partial sums (scalar Square grouped with h2 squares;
        # only needed in the tail). sum over everything = 4*sum|Zp|^2.
        zpsq = sb.tile([64, 2], f32, name="zpsq")
        for t in range(2):
            sq2 = sb3.tile([64, BTR], bf16, tag="zpsq_scr", name="zpsq_scr")
            nc.scalar.activation(out=sq2[:], in_=zp8[:64, t, :],
                                 func=AF.Square, accum_out=zpsq[:, t:t + 1])

        # ---------------- AGa: z-gram partials + h2 moment partials
        aga_in = dram.tile([128, AGAF], f32, name="aga_in")
        nc.sync.dma_start(out=aga_in[:], in_=paya[:])
        nc.gpsimd.collective_compute(
            "AllGather", ALU.bypass, ins=[aga_in[:].opt()],
            outs=[aga_out[:].opt()], replica_groups=[list(range(NCORES))])

        # ---------------- NCT distance loop (fp8 DR dots incl. nsq row)
        payb = sb.tile([128, AGBF], f32, name="payb")
        dmin32 = sb.tile([128, 2 * NICH], f32, name="dmin32")
        for ic in range(NICH):
            lhs = zp8[:, :, ic * 128:(ic + 1) * 128]
            for jh in range(2):
                dps = ps_d.tile([128, 1024], f32, tag="dps", name="dps")
                for jq in range(2):
                    off = jh * 1024 + jq * 512
                    nc.tensor.matmul(out=dps[:, jq * 512:(jq + 1) * 512],
                                     lhsT=lhs,
                                     rhs=z8[:, :, off:off + 512],
                                     start=True, stop=True, perf_mode=DR)
                col = ic * 2 + jh
                nc.vector.tensor_reduce(out=dmin32[:, col:col + 1],
                                        in_=dps[:], axis=AX.X, op=ALU.min)
        dmv = dmin32[:].rearrange("p (i h) -> p i h", h=2)
        nc.vector.tensor_tensor(out=payb[:, NADD:AGBF], in0=dmv[:, :, 0],
                                in1=dmv[:, :, 1], op=ALU.min)

        # ---------------- AGa combine (gpsimd) -> glo stats + h2 stats
        agal = sb.tile([128, NCORES, AGAF], f32, name="agal")
        nc.sync.dma_start(out=agal[:],
                          in_=aga_out[:].rearrange("(c p) f -> p c f", p=128))
        suma = sb.tile([128, AGAF], f32, name="suma")
        nc.vector.tensor_tensor(out=suma[:], in0=agal[:, 0, :],
                                in1=agal[:, 1, :], op=ALU.add)
        for c in range(2, NCORES):
            nc.vector.tensor_tensor(out=suma[:], in0=suma[:],
                                    in1=agal[:, c, :], op=ALU.add)

        # h2 stats (on [128, 2] cols)
        h2tot = sb.tile([128, 2], f32, name="h2tot")
        h2tot2 = sb.tile([128, 2], f32, name="h2tot2")
        base = LAT + 1
        sv1 = suma[:, base:base + 4].rearrange("p (b n) -> p b n", n=2)
        sv2 = suma[:, base + 4:base + 8].rearrange("p (b n) -> p b n", n=2)
        nc.vector.tensor_tensor(out=h2tot[:], in0=sv1[:, :, 0],
                                in1=sv1[:, :, 1], op=ALU.add)
        nc.vector.tensor_tensor(out=h2tot2[:], in0=sv2[:, :, 0],
                                in1=sv2[:, :, 1], op=ALU.add)
        h2mu = sb.tile([128, 2], f32, name="h2mu")
        nc.scalar.activation(out=h2mu[:], in_=h2tot[:], func=AF.Copy,
                             scale=1.0 / NIND)
        h2_s, h2_b = _stat_tail(h2tot2[:], h2mu[:], g_gam, g_bet, NIND, "h2")
        # glo stats from global z gram
        glo_s, glo_b = stats_from_gram(suma[:, 0:LAT + 1], gW1T_32, gW1nat,
                                       g_gam, g_bet, LAT, NS, "glo")

        # ---------------- ind tail: h2a -> xiT -> praw/colsum (AGb critical)
        h2a = [sb.tile([128, SH_NI], f32r, tag=f"h2a{b}", name=f"h2a{b}")
               for b in range(2)]
        for b in range(2):
            nc.scalar.activation(out=h2a[b][:], in_=h2[b][:], func=AF.Lrelu,
                                 bias=h2_b[:, b:b + 1], scale=h2_s[:, b:b + 1],
                                 alpha=LRELU)
        xiT = sb.tile([SIZE, SH_NI], f32r, name="xiT")
        cs2 = sb.tile([SIZE, 2], f32, name="cs2")
        for n in range(SH_NI // 512):
            xp = ps_sm.tile([SIZE, 512], f32, tag="sm", name="xi_mm")
            for b in range(2):
                nc.tensor.matmul(out=xp[:],
                                 lhsT=gW2T_r[:, b * 64:(b + 1) * 64],
                                 rhs=h2a[b][:, n * 512:(n + 1) * 512],
                                 start=(b == 0), stop=(b == 1))
            nc.scalar.activation(out=xiT[:, n * 512:(n + 1) * 512], in_=xp[:],
                                 func=AF.Identity, bias=g_b2[:],
                                 accum_out=cs2[:, n:n + 1])
        nc.vector.tensor_tensor(out=payb[:SIZE, SIZE:SIZE + 1],
                                in0=cs2[:, 0:1], in1=cs2[:, 1:2], op=ALU.add)
        xin = sb.tile([128, SH_NI // 128, SIZE], f32r, name="xin")
        for g in range(SH_NI // 128):
            tp = ps_sm.tile([128, SIZE], f32r, tag="sm", name="xi_tp")
            nc.tensor.transpose(out=tp[:], in_=xiT[:, g * 128:(g + 1) * 128],
                                identity=identr[:SIZE, :SIZE])
            nc.scalar.copy(out=xin[:, g, :], in_=tp[:])
        praw = ps_acc.tile([SIZE, SIZE], f32, tag="acc", name="praw")
        for g in range(SH_NI // 128):
            nc.tensor.matmul(out=praw[:], lhsT=xin[:, g, :], rhs=xin[:, g, :],
                             start=(g == 0), stop=(g == SH_NI // 128 - 1))
        nc.scalar.copy(out=payb[:SIZE, 0:SIZE], in_=praw[:])

        # ---------------- glo branch -> mse (parallel to ind tail)
        h8glo = sb.tile([128, 2, SH_NS], fp8, name="h8glo")
        for b in range(2):
            for n in range(SH_NS // 512):
                hp = ps_sm.tile([128, 512], f32, tag="sm", name="hg_mm")
                nc.tensor.matmul(out=hp[:],
                                 lhsT=gW1T8[:, :, b * 128:(b + 1) * 128],
                                 rhs=z8[:64, :, n * 512:(n + 1) * 512],
                                 start=True, stop=True, perf_mode=DR)
                nc.scalar.activation(out=h8glo[:, b, n * 512:(n + 1) * 512],
                                     in_=hp[:], func=AF.Lrelu,
                                     bias=glo_b[:, b:b + 1],
                                     scale=glo_s[:, b:b + 1], alpha=LRELU)
        msec = sb.tile([SIZE, SH_NS // 512], f32, name="msec")
        for n in range(SH_NS // 512):
            xp = ps_sm.tile([SIZE, 512], f32, tag="sm", name="xg_mm")
            nc.tensor.matmul(out=xp[:], lhsT=gW2T8[:],
                             rhs=h8glo[:, :, n * 512:(n + 1) * 512],
                             start=True, stop=True, perf_mode=DR)
            dsc = sb3.tile([SIZE, 512], bf16, tag="mse_scr", name="mse_scr")
            nc.vector.scalar_tensor_tensor(
                out=dsc[:], in0=xp[:], scalar=g_b2[:],
                in1=t_xT[:, n * 512:(n + 1) * 512],
                op0=ALU.add, op1=ALU.subtract)
            sqm = sb3.tile([SIZE, 512], bf16, tag="mse_scr2", name="mse_scr2")
            nc.scalar.activation(out=sqm[:], in_=dsc[:], func=AF.Square,
                                 accum_out=msec[:, n:n + 1])
        mse2 = sb.tile([SIZE, 2], f32, name="mse2")
        msev = msec[:].rearrange("p (a b) -> p a b", a=2)
        nc.vector.tensor_tensor(out=mse2[:], in0=msev[:, :, 0],
                                in1=msev[:, :, 1], op=ALU.add)
        nc.vector.tensor_tensor(out=payb[:SIZE, SIZE + 1:SIZE + 2],
                                in0=mse2[:, 0:1], in1=mse2[:, 1:2],
                                op=ALU.add)

        # ---------------- AGb
        agb_in = dram.tile([128, AGBF], f32, name="agb_in")
        nc.sync.dma_start(out=agb_in[:], in_=payb[:])
        nc.gpsimd.collective_compute(
            "AllGather", ALU.bypass, ins=[agb_in[:].opt()],
            outs=[agb_out[:].opt()], replica_groups=[list(range(NCORES))])
        agbl = sb.tile([128, NCORES, AGBF], f32, name="agbl")
        nc.sync.dma_start(out=agbl[:],
                          in_=agb_out[:].rearrange("(c p) f -> p c f", p=128))
        sumb = sb.tile([128, NADD], f32, name="sumb")
        nc.vector.tensor_tensor(out=sumb[:], in0=agbl[:, 0, 0:NADD],
                                in1=agbl[:, 1, 0:NADD], op=ALU.add)
        for c in range(2, NCORES):
            nc.vector.tensor_tensor(out=sumb[:], in0=sumb[:],
                                    in1=agbl[:, c, 0:NADD], op=ALU.add)
        dmin = sb.tile([128, NICH], f32, name="dmin")
        nc.vector.tensor_tensor(out=dmin[:], in0=agbl[:, 0, NADD:AGBF],
                                in1=agbl[:, 1, NADD:AGBF], op=ALU.min)
        for c in range(2, NCORES):
            nc.vector.tensor_tensor(out=dmin[:], in0=dmin[:],
                                    in1=agbl[:, c, NADD:AGBF], op=ALU.min)
        dsum = sb.tile([128, 1], f32, name="dsum")
        nc.vector.reduce_sum(out=dsum[:], in_=dmin[:], axis=AX.X)

        # ---------------- loss_indep final math ([64, 64] fp32)
        S64 = SIZE

        def new64(tag):
            return sb.tile([S64, S64], f32, tag=tag, name=tag)

        csum = sb.tile([S64, 1], f32, name="csum")
        nc.vector.tensor_copy(out=csum[:], in_=sumb[:S64, S64:S64 + 1])
        cr_ps = ps_sm.tile([1, S64], f32, tag="sm", name="cr_ps")
        nc.tensor.transpose(out=cr_ps[:], in_=csum[:],
                            identity=ident_32[:S64, :S64])
        csr = sb.tile([1, S64], f32, name="csr")
        nc.scalar.copy(out=csr[:], in_=cr_ps[:])
        mr = sb.tile([1, S64], f32, name="mr")
        nc.scalar.activation(out=mr[:], in_=csr[:], func=AF.Copy,
                             scale=1.0 / NIND)
        outer_ps = ps_sm.tile([S64, S64], f32, tag="sm", name="outer_ps")
        nc.tensor.matmul(out=outer_ps[:], lhsT=mr[:], rhs=csr[:],
                         start=True, stop=True)
        S_t = new64("S_t")
        nc.vector.tensor_tensor(out=S_t[:], in0=sumb[:S64, 0:S64],
                                in1=outer_ps[:], op=ALU.subtract)
        dtmp = new64("dtmp")
        nc.vector.tensor_tensor(out=dtmp[:], in0=S_t[:], in1=eye[:],
                                op=ALU.mult)
        s2 = sb.tile([S64, 1], f32, name="s2")
        nc.vector.reduce_sum(out=s2[:], in_=dtmp[:], axis=AX.X)
        r2 = sb.tile([S64, 1], f32, name="r2")
        nc.vector.reciprocal(out=r2[:], in_=s2[:])
        s2r_ps = ps_sm.tile([1, S64], f32, tag="sm", name="s2r_ps")
        nc.tensor.transpose(out=s2r_ps[:], in_=s2[:],
                            identity=ident_32[:S64, :S64])
        s2row = sb.tile([1, S64], f32, name="s2row")
        nc.scalar.copy(out=s2row[:], in_=s2r_ps[:])
        onesr64 = sb.tile([1, S64], f32, name="onesr64")
        nc.vector.memset(onesr64[:], 1.0)
        s2b_ps = ps_sm.tile([S64, S64], f32, tag="sm", name="s2b_ps")
        nc.tensor.matmul(out=s2b_ps[:], lhsT=onesr64[:], rhs=s2row[:],
                         start=True, stop=True)
        s2b = new64("s2b")
        nc.scalar.copy(out=s2b[:], in_=s2b_ps[:])
        SS = new64("SS")
        nc.vector.tensor_tensor(out=SS[:], in0=S_t[:], in1=S_t[:],
                                op=ALU.mult)
        F_t = new64("F_t")
        nc.vector.tensor_scalar_mul(out=F_t[:], in0=SS[:], scalar1=r2[:])
        dg = new64("dg")
        nc.vector.tensor_tensor(out=dg[:], in0=s2b[:], in1=F_t[:],
                                op=ALU.subtract)
        nc.vector.tensor_tensor(out=dg[:], in0=dg[:], in1=eye[:], op=ALU.add)
        B_t = new64("B_t")
        nc.vector.reciprocal(out=B_t[:], in_=dg[:])
        nc.vector.tensor_tensor(out=B_t[:], in0=B_t[:], in1=offd[:],
                                op=ALU.mult)
        P_t = new64("P_t")
        nc.vector.tensor_tensor(out=P_t[:], in0=U_t[:], in1=B_t[:],
                                op=ALU.mult)
        Q_t = new64("Q_t")
        nc.vector.tensor_tensor(out=Q_t[:], in0=C_t[:], in1=B_t[:],
                                op=ALU.mult)
        ptq_ps = ps_sm.tile([S64, S64], f32, tag="sm", name="ptq_ps")
        nc.tensor.matmul(out=ptq_ps[:], lhsT=P_t[:], rhs=Q_t[:],
                         start=True, stop=True)
        t1_t = new64("t1_t")
        nc.vector.tensor_tensor(out=t1_t[:], in0=SS[:], in1=ptq_ps[:],
                                op=ALU.mult)
        nc.vector.reduce_sum(out=fin64[:, 1:2], in_=t1_t[:], axis=AX.X)
        A_t = new64("A_t")
        nc.vector.tensor_tensor(out=A_t[:], in0=P_t[:], in1=S_t[:],
                                op=ALU.mult)
        Bt_t = new64("Bt_t")
        nc.vector.tensor_tensor(out=Bt_t[:], in0=Q_t[:], in1=S_t[:],
                                op=ALU.mult)
        nc.vector.tensor_scalar_mul(out=Bt_t[:], in0=Bt_t[:], scalar1=r2[:])
        ab_ps = ps_sm.tile([S64, S64], f32, tag="sm", name="ab_ps")
        nc.tensor.matmul(out=ab_ps[:], lhsT=A_t[:], rhs=Bt_t[:],
                         start=True, stop=True)
        t2_t = new64("t2_t")
        nc.vector.tensor_tensor(out=t2_t[:], in0=S_t[:], in1=ab_ps[:],
                                op=ALU.mult)
        nc.vector.reduce_sum(out=fin64[:, 2:3], in_=t2_t[:], axis=AX.X)
        g1 = new64("t1_t")
        nc.vector.tensor_tensor(out=g1[:], in0=P_t[:], in1=SS[:],
                                op=ALU.mult)
        gc = sb.tile([S64, 1], f32, tag="gcol", name="gcol")
        nc.vector.reduce_sum(out=gc[:], in_=g1[:], axis=AX.X)
        d1 = new64("t2_t")
        nc.vector.tensor_tensor(out=d1[:], in0=Q_t[:], in1=SS[:],
                                op=ALU.mult)
        dc = sb.tile([S64, 1], f32, tag="dcol", name="dcol")
        nc.vector.reduce_sum(out=dc[:], in_=d1[:], axis=AX.X)
        t3c = sb.tile([S64, 1], f32, tag="t3col", name="t3col")
        nc.vector.tensor_tensor(out=t3c[:], in0=gc[:], in1=dc[:], op=ALU.mult)
        nc.vector.tensor_tensor(out=t3c[:], in0=t3c[:], in1=r2[:],
                                op=ALU.mult)
        nc.vector.tensor_tensor(out=t3c[:], in0=t3c[:], in1=r2[:],
                                op=ALU.mult)
        nc.vector.tensor_copy(out=fin64[:, 3:4], in_=t3c[:])
        r2b = new64("dtmp")
        nc.vector.reciprocal(out=r2b[:], in_=s2b[:])
        ss_t = new64("t1_t")
        nc.vector.tensor_tensor(out=ss_t[:], in0=F_t[:], in1=r2b[:],
                                op=ALU.mult)
        nc.vector.tensor_tensor(out=ss_t[:], in0=ss_t[:], in1=offd[:],
                                op=ALU.mult)
        nc.vector.reduce_sum(out=fin64[:, 5:6], in_=ss_t[:], axis=AX.X)
        nc.vector.tensor_copy(out=fin64[:, 6:7], in_=sumb[:S64, S64 + 1:])
        nc.vector.tensor_tensor(out=fin64[:, 7:8], in0=zpsq[:, 0:1],
                                in1=zpsq[:, 1:2], op=ALU.add)

        f64_ps = ps_sm.tile([1, 8], f32, tag="sm", name="f64_ps")
        nc.tensor.matmul(out=f64_ps[:], lhsT=ones64[:], rhs=fin64[:],
                         start=True, stop=True)
        frow = sb.tile([1, 8], f32, name="frow")
        nc.scalar.copy(out=frow[:], in_=f64_ps[:])
        f128_ps = ps_sm.tile([1, 1], f32, tag="sm", name="f128_ps")
        nc.tensor.matmul(out=f128_ps[:], lhsT=ones128[:], rhs=dsum[:],
                         start=True, stop=True)
        grow = sb.tile([1, 1], f32, name="grow")
        nc.scalar.copy(out=grow[:], in_=f128_ps[:])

        acc = sb.tile([1, 1], f32, name="acc_sc")
        tmp = sb.tile([1, 1], f32, tag="tmp_sc", name="tmp_sc")
        nc.vector.tensor_copy(out=acc[:], in_=frow[:, 0:1])       # loss_trans
        nc.scalar.activation(out=tmp[:], in_=frow[:, 6:7], func=AF.Copy,
                             scale=1.0 / (NS * SIZE))             # mse
        nc.vector.tensor_tensor(out=acc[:], in0=acc[:], in1=tmp[:],
                                op=ALU.add)
        nc.scalar.activation(out=tmp[:], in_=grow[:, 0:1], func=AF.Copy,
                             scale=1.0 / (BTR * LAT))             # nct mins
        nc.vector.tensor_tensor(out=acc[:], in0=acc[:], in1=tmp[:],
                                op=ALU.add)
        nc.scalar.activation(out=tmp[:], in_=frow[:, 7:8], func=AF.Copy,
                             scale=0.25 / (BTR * LAT))            # nct |Zp|^2
        nc.vector.tensor_tensor(out=acc[:], in0=acc[:], in1=tmp[:],
                                op=ALU.add)
        nc.vector.tensor_tensor(out=acc[:], in0=acc[:], in1=frow[:, 1:2],
                                op=ALU.add)                       # t1
        nc.scalar.activation(out=tmp[:], in_=frow[:, 2:3], func=AF.Copy,
                             scale=-2.0)                          # -2 t2
        nc.vector.tensor_tensor(out=acc[:], in0=acc[:], in1=tmp[:],
                                op=ALU.add)
        nc.vector.tensor_tensor(out=acc[:], in0=acc[:], in1=frow[:, 3:4],
                                op=ALU.add)                       # t3
        nc.vector.tensor_tensor(out=acc[:], in0=acc[:], in1=frow[:, 4:5],
                                op=ALU.subtract)                  # -t4
        nc.scalar.activation(out=tmp[:], in_=frow[:, 5:6], func=AF.Copy,
                             scale=float(S64 - 2))                # sum_sc
        nc.vector.tensor_tensor(out=acc[:], in0=acc[:], in1=tmp[:],
                                op=ALU.add)
        nc.sync.dma_start(out=out_d[:], in_=acc[:])

    _split_multi_waits(nc)
    return nc


def _dr_lhsT(wT, ksplit):
    """[K, M] -> [K//2, 2, M] DoubleRow stationary layout."""
    K, M = wT.shape
    assert K == 2 * ksplit
    return np.ascontiguousarray(
        wT.reshape(2, ksplit, M).transpose(1, 0, 2))


def _stage_inputs(I):
    g = lambda k: np.asarray(I[k], dtype=np.float32)
    z = g("z_logits")
    X = g("X")
    ntr = g("noise_trans")
    nind = g("noise_indep")
    L = g("conn_logits")

    def bf(a):
        return np.ascontiguousarray(a.astype(bfnp))

    def h16(a):
        return np.ascontiguousarray(a.astype(np.float16))

    def f8(a):
        return np.ascontiguousarray(a.astype(f8np))

    def f(a):
        return np.ascontiguousarray(a.astype(np.float32))

    c8_blob = np.zeros((128, C8W), f8np)
    c32_blob = np.zeros((128, C32W), np.float32)
    cfr_blob = np.zeros((128, CFRW), np.float32)

    def put(blob, m, name, arr):
        r, c0, w = m[name]
        blob[:r, c0:c0 + w] = arr.reshape(r, w).astype(blob.dtype)

    put(c8_blob, C8_MAP, "gW1T8", _dr_lhsT(g("glo_W1").T, 64))
    put(c8_blob, C8_MAP, "gW2T8", _dr_lhsT(g("glo_W2").T, 128))
    put(c8_blob, C8_MAP, "tW1T8", _dr_lhsT(g("tr_W1").T, 32))
    put(c8_blob, C8_MAP, "tW2T8", _dr_lhsT(g("tr_W2").T, 128))
    put(c32_blob, C32_MAP, "ident_32", np.eye(128, dtype=np.float32))
    put(c32_blob, C32_MAP, "eye", np.eye(SIZE, dtype=np.float32))
    put(c32_blob, C32_MAP, "offd", 1.0 - np.eye(SIZE, dtype=np.float32))
    put(c32_blob, C32_MAP, "L", L)
    put(c32_blob, C32_MAP, "LT", L.T)
    put(c32_blob, C32_MAP, "g_gam", g("glo_gamma").reshape(2, 128).T)
    put(c32_blob, C32_MAP, "g_bet", g("glo_beta").reshape(2, 128).T)
    put(c32_blob, C32_MAP, "t_gam", g("tr_gamma").reshape(2, 128).T)
    put(c32_blob, C32_MAP, "t_bet", g("tr_beta").reshape(2, 128).T)
    put(c32_blob, C32_MAP, "g_b2", g("glo_b2").reshape(-1, 1))
    put(c32_blob, C32_MAP, "t_b2", g("tr_b2").reshape(-1, 1))
    put(c32_blob, C32_MAP, "tb2m2", -2.0 * g("tr_b2").reshape(2, 64).T)
    put(c32_blob, C32_MAP, "ones64", np.ones((SIZE, 1), np.float32))
    put(c32_blob, C32_MAP, "ones128", np.ones((128, 1), np.float32))
    put(c32_blob, C32_MAP, "tW1T_32", g("tr_W1").T)
    put(c32_blob, C32_MAP, "gW1T_32", g("glo_W1").T)
    put(c32_blob, C32_MAP, "tW1nat",
        np.concatenate([g("tr_W1")[:128], g("tr_W1")[128:]], axis=1))
    put(c32_blob, C32_MAP, "gW1nat",
        np.concatenate([g("glo_W1")[:128], g("glo_W1")[128:]], axis=1))
    put(cfr_blob, CFR_MAP, "identr", np.eye(128, dtype=np.float32))
    put(cfr_blob, CFR_MAP, "tW1T_r", g("tr_W1").T)
    put(cfr_blob, CFR_MAP, "tW2T_r", g("tr_W2").T)
    put(cfr_blob, CFR_MAP, "gW1T_r", g("glo_W1").T)
    put(cfr_blob, CFR_MAP, "gW2T_r", g("glo_W2").T)

    zprow = np.zeros((1, 2 * BTR), np.float32)
    zprow[0, :BTR] = 1.0
    shared = {
        "ntr16": h16(np.concatenate([ntr, np.ones((BTR, 1), np.float32)], 1)),
        "ntrT8": f8(ntr.T.reshape(2, 32, BTR).transpose(1, 0, 2)),
        "zprow": f8(zprow),
        "nind16": h16(np.concatenate(
            [nind, np.ones((NIND, 1), np.float32)], 1)),
        "c8": c8_blob, "c32": c32_blob, "cfr": cfr_blob,
    }
    zT = z.T
    XT = X.T
    nindT = nind.T
    maps = []
    for c in range(NCORES):
        m = dict(shared)
        zsh = z[c * SH_NS:(c + 1) * SH_NS, :]
        m["znat16"] = h16(np.concatenate(
            [zsh, np.ones((SH_NS, 1), np.float32)], 1))
        z8a = np.zeros((65, 2, SH_NS), np.float32)
        z8a[:64] = (zT[:, c * SH_NS:(c + 1) * SH_NS]
                    .reshape(2, 64, SH_NS).transpose(1, 0, 2))
        m["z8d"] = f8(z8a)
        m["xT_sh"] = bf(XT[:, c * SH_NS:(c + 1) * SH_NS])
        m["nindT32"] = f(nindT[:, c * SH_NI:(c + 1) * SH_NI])
        maps.append(m)
    return maps


def _get_nc():
    if "nc" not in _CACHE:
        _install_profshim()
        _CACHE["nc"] = _build_program()
    return _CACHE["nc"]


def run(inputs, trace=False):
    nc = _get_nc()
    maps = _stage_inputs(inputs)
    res = run_bass_kernel_spmd(nc, maps, list(range(NCORES)), trace=trace)
    val = np.float32(res.results[0]["out"].reshape(-1)[0])
    return val, res


def kernel(**inputs) -> np.ndarray:
    val, _ = run(inputs, trace=False)
    return np.asarray(val, dtype=np.float32)


if __name__ == "__main__":
    nc = _get_nc()
    ninst = sum(len(bb.instructions) for bb in nc.main_func.blocks)
    print("built ok, instructions:", ninst)
